# revision 74
# baseline (speedup 1.0000x reference)
"""Self-contained Trainium2 Bass kernel for the nn_EnocoderBlock problem.

kernel(**inputs) takes the full (unsharded) inputs of the reference encoder
block (B=2, S=2048, D=1024, H=16, DFF=4096) and returns the full [B, S, D]
fp32 output, running SPMD on 8 NeuronCores.

Sharding: data-parallel over batch x query-token blocks — each of the 8
cores owns one batch element's full K/V context and a 512-token query
slice, so no cross-core collectives are needed.

Precision: all large GEMMs run in fp8e4m3 with DoubleRow perf mode (0.5
PE cycles per output column).  The QK^T scores (64-deep contraction) use
DoubleRow with a zero second slot in the moving operand.  FFN weights
and activations are split into fp8 hi+lo pairs (error compensation), so
the end-to-end error stays ~2e-3.  Scale factors fold into weights / the
exp bias / LayerNorm constants (LayerNorm is scale-invariant).

Schedule: softmax exp on the Activation engine is the critical resource
(~133us).  Attention runs in two query halves; the exp stream starts as
early as possible and everything else (V/K/Q projections in half A; the
O-projection, LayerNorm1, transposes and FFN1 of half A inside half B's
window) is interleaved into the exp-bound windows as PE/DVE/Pool filler.
LayerNorms run DVE-only (affine_mul_reduce) to keep Act pure-exp.
"""

import sys
for _p in ("/opt/trn_rl_repo", "/root/.axon_site/_ro/trn_rl_repo"):
    if _p not in sys.path:
        sys.path.append(_p)

import numpy as np

import math
from contextlib import ExitStack

import concourse.mybir as mybir
import concourse.tile as tile
from concourse.bass import ds, ts

F32 = mybir.dt.float32
BF16 = mybir.dt.bfloat16
FP8 = mybir.dt.float8e4
AX = mybir.AxisListType
ALU = mybir.AluOpType
ACTF = mybir.ActivationFunctionType
DR = mybir.MatmulPerfMode.DoubleRow

P = 128
EPS = 1e-6
LNC = math.log(4.0)       # exp scale constant folded into activation bias
S_RES = 1024.0            # attention residual pre-scale (ctx32 @ wo32)
S_FF = 256.0              # ffn residual pre-scale (hid16 @ w216)


def build(nc, S=2048, D=1024, H=16, DK=64, DFF=4096, TQ=512):
    assert DK == 64 and D % P == 0 and S % P == 0 and DFF % P == 0
    NJ = D // P            # feature tiles of 128 (8)
    NT = S // P            # token tiles of 128 (16)
    NTQ = TQ // P          # query token tiles of 128 (4)
    NF = DFF // P          # dff tiles of 128 (32)
    HPJ = P // DK          # heads per 128-feature tile (2)
    HG = 2                 # attention head-group size
    TN = 512               # moving-dim tile (tokens)
    NTN = S // TN          # 4
    MQ = TQ // 2           # query-half width (256)
    NQ4 = NT // 4          # score quads per head per half (4)

    # ---------------- DRAM I/O ----------------
    def din(name, shape, dt):
        return nc.dram_tensor(name, shape, dt, kind="ExternalInput").ap()

    xT8 = din("xT8", [D, S], FP8)
    xTq8 = din("xTq8", [D, TQ], FP8)
    xqb = din("xqb", [TQ, D], BF16)           # S_RES * (x_q + bo + bv@wo^T)
    wv8, wk8 = din("wv8", [D, D], FP8), din("wk8", [D, D], FP8)
    wq8, wo8 = din("wq8", [D, D], FP8), din("wo8", [D, D], FP8)
    w1hi, w1lo = din("w1hi", [D, DFF], FP8), din("w1lo", [D, DFF], FP8)
    w2hi, w2lo = din("w2hi", [DFF, D], FP8), din("w2lo", [DFF, D], FP8)
    bq, bk = din("bq", [D], F32), din("bk", [D], F32)
    b1, b2 = din("b1", [DFF], F32), din("b2", [D], F32)
    alpha, gamma = din("alpha", [1], F32), din("gamma", [1], F32)
    out = nc.dram_tensor("out", [TQ, D], F32, kind="ExternalOutput").ap()

    xT_v = xT8.rearrange("(o p) t -> p o t", p=P)
    xTq_v = xTq8.rearrange("(o p) t -> p o t", p=P)
    xqb_v = xqb.rearrange("(o p) d -> p o d", p=P)
    out_v = out.rearrange("(o p) d -> p o d", p=P)
    wv_v = wv8.rearrange("(o p) j -> p o j", p=P)
    wk_v = wk8.rearrange("(o p) j -> p o j", p=P)
    wq_v = wq8.rearrange("(o p) j -> p o j", p=P)
    wo_v = wo8.rearrange("(o p) j -> p o j", p=P)
    w1hi_v = w1hi.rearrange("(o p) f -> p o f", p=P)
    w1lo_v = w1lo.rearrange("(o p) f -> p o f", p=P)
    w2hi_v = w2hi.rearrange("(o p) j -> p o j", p=P)
    w2lo_v = w2lo.rearrange("(o p) j -> p o j", p=P)
    bq_v = bq.rearrange("(o p) -> p o", p=P)
    bk_v = bk.rearrange("(o p) -> p o", p=P)
    b1_v = b1.rearrange("(o p) -> p o", p=P)

    with tile.TileContext(nc) as tc, ExitStack() as octx:
        small = octx.enter_context(tc.tile_pool(name="small", bufs=1))

        # ============ pools (LIFO; xtwp dies mid-A, kqv at B-end) ========
        ctx2_cm = tc.tile_pool(name="ctx2", bufs=1)
        ctx2 = ctx2_cm.__enter__()
        ctx_sb = ctx2.tile([P, NJ, TQ], FP8, tag="ctx")
        wo_sb = ctx2.tile([P, NJ, D], FP8, tag="wo")
        xqb_sb = ctx2.tile([P, NTQ, D], BF16, tag="xqb")

        late_cm = tc.tile_pool(name="late", bufs=1)
        late = late_cm.__enter__()
        out1_sb = late.tile([P, NTQ, D], BF16, tag="out1")   # 256*out1
        out1T8 = late.tile([P, NJ, TQ], FP8, tag="out1T")
        out1T8l = late.tile([P, NJ, TQ], FP8, tag="out1Tl")
        hid_sb = late.tile([P, NF, TQ], FP8, tag="hid")      # 16*relu hi
        hid_lo = late.tile([P, NF, TQ], FP8, tag="hidlo")
        res2_sb = late.tile([P, NTQ, D], F32, tag="res2")

        fs_cm = tc.tile_pool(name="fstream", bufs=2)
        fstream = fs_cm.__enter__()

        kqv_cm = tc.tile_pool(name="kqv", bufs=1)
        kqv = kqv_cm.__enter__()
        # K has a zeroed 128-token tail: the DR scores lhsT uses 2 token-
        # tile slots and slot 1 (multiplying Q2's zero slot) must be finite
        K_sb = kqv.tile([P, NJ, S + P], FP8, tag="K")
        Q2_sb = kqv.tile([P, NJ, 2, TQ], FP8, tag="Q2")      # [Q; 0] pairs
        V_sb = kqv.tile([P, NT, H, DK + 1], FP8, tag="V")

        dp_cm = tc.tile_pool(name="dpool", bufs=2)
        dpool = dp_cm.__enter__()

        xtwp_cm = tc.tile_pool(name="xtwp", bufs=1)
        xtwp = xtwp_cm.__enter__()
        xt_all = xtwp.tile([P, NJ, S], FP8, tag="xt")
        wv_sb = xtwp.tile([P, NJ, D], FP8, tag="wv")
        wk_sb = xtwp.tile([P, NJ, D], FP8, tag="wk")
        wq_sb = xtwp.tile([P, NJ, D], FP8, tag="wq")
        xTq_sb = xtwp.tile([P, NJ, TQ], FP8, tag="xTq")

        # ---- input DMAs, ordered for streaming ----
        nc.sync.dma_start(wv_sb[:], wv_v)
        bq_sb = small.tile([P, NJ], F32, tag="bq")
        bk_sb = small.tile([P, NJ], F32, tag="bk")
        XC = 512
        for c in range(S // XC):
            nc.sync.dma_start(xt_all[:, :, ds(c * XC, XC)],
                              xT_v[:, :, ds(c * XC, XC)])
            if c == 1:
                nc.sync.dma_start(wk_sb[:], wk_v)
        nc.sync.dma_start(bk_sb[:], bk_v)
        nc.sync.dma_start(bq_sb[:], bq_v)
        nc.sync.dma_start(wq_sb[:], wq_v)
        nc.sync.dma_start(xTq_sb[:], xTq_v)
        nc.gpsimd.memset(Q2_sb[:, :, 1, :], 0.0)
        nc.gpsimd.memset(K_sb[:, :, S:], 0.0)
        nc.vector.memset(V_sb[:, :, :, DK:DK + 1], 1.0)

        # ---------------- constants / biases ----------------
        b1_sb = small.tile([P, NF], F32, tag="b1")
        nc.sync.dma_start(b1_sb[:], b1_v)
        b1x16 = small.tile([P, NF], F32, tag="b1x16")
        nc.vector.tensor_scalar_mul(b1x16[:], b1_sb[:], 16.0)

        # row staging goes through partition 0 of the (not-yet-used) LN
        # squares-dump tile — SBUF is too tight for a dedicated rows pool
        sqd_sb = late.tile([P, D], F32, tag="sqd")
        nc.sync.dma_start(sqd_sb[0:1, :], b2[None, :])
        b2s_bc = small.tile([P, D], BF16, tag="b2s_bc")
        nc.vector.tensor_scalar_mul(b2s_bc[0:1, :], sqd_sb[0:1, :], S_FF)
        nc.gpsimd.partition_broadcast(b2s_bc[:], b2s_bc[0:1, :])

        ag_st = late.tile([P, 8], F32, tag="lnstat", bufs=2)
        nc.sync.dma_start(ag_st[0:1, 0:1], alpha[None, :])
        nc.sync.dma_start(ag_st[0:1, 1:2], gamma[None, :])
        ag_bc = small.tile([P, 2], F32, tag="ag_bc")
        nc.gpsimd.partition_broadcast(ag_bc[:], ag_st[0:1, 0:2])
        alpha_bc = ag_bc[:, 0:1]
        gamma_bc = ag_bc[:, 1:2]
        ag256 = small.tile([P, 2], F32, tag="ag256")
        nc.vector.tensor_scalar_mul(ag256[:], ag_bc[:], S_FF)
        alpha256_bc = ag256[:, 0:1]
        gamma256_bc = ag256[:, 1:2]

        eps_bc = small.tile([P, 1], F32, tag="eps_bc")
        nc.vector.memset(eps_bc[:], EPS)
        lnc_bc = small.tile([P, 1], F32, tag="lnc_bc")
        nc.vector.memset(lnc_bc[:], LNC)

        nc.sync.dma_start(wo_sb[:], wo_v)
        nc.sync.dma_start(xqb_sb[:], xqb_v)

        # ================= PSUM pools (8 banks total) =================
        psd_cm = tc.tile_pool(name="psd", bufs=2, space="PSUM")
        psd = psd_cm.__enter__()        # ps4 [P,4,MQ] f32 = 2 banks x2
        psc_cm = tc.tile_pool(name="psc", bufs=2, space="PSUM")
        psc = psc_cm.__enter__()        # c2 [P,512] f32 = 1 bank x2
        pse_cm = tc.tile_pool(name="pse", bufs=1, space="PSUM")
        pse = pse_cm.__enter__()        # pe [P,512] f32 + pst [P,1024] bf16

        # ---------------- filler helpers ----------------
        def v_tiles(tt0, n):
            """V projection for token tiles tt0..tt0+n-1 (32*v, no bias)."""
            VN = 512
            for tt in range(tt0, tt0 + n):
                for nv in range(D // VN):
                    ps = pse.tile([P, VN], F32, tag="pe")
                    for kk in range(NJ // 2):
                        nc.tensor.matmul(
                            ps[:], xt_all[:, ds(2 * kk, 2), ts(tt, P)],
                            wv_sb[:, ds(2 * kk, 2), ds(nv * VN, VN)],
                            start=(kk == 0), stop=(kk == NJ // 2 - 1),
                            perf_mode=DR)
                    nc.vector.tensor_copy(
                        V_sb[:, tt, ds(nv * (VN // DK), VN // DK), 0:DK],
                        ps[:].rearrange("p (h d) -> p h d", d=DK))

        def kq_proj(jt):
            for nt in range(NTN):
                ps = pse.tile([P, TN], F32, tag="pe")
                for kk in range(NJ // 2):
                    nc.tensor.matmul(
                        ps[:], wk_sb[:, ds(2 * kk, 2), ts(jt, P)],
                        xt_all[:, ds(2 * kk, 2), ds(nt * TN, TN)],
                        start=(kk == 0), stop=(kk == NJ // 2 - 1),
                        perf_mode=DR)
                nc.vector.tensor_scalar(
                    K_sb[:, jt, ds(nt * TN, TN)], ps[:],
                    1.0 / 16.0, bk_sb[:, jt:jt + 1], ALU.mult, ALU.add)
            ps = pse.tile([P, TQ], F32, tag="pe")
            for kk in range(NJ // 2):
                nc.tensor.matmul(
                    ps[:], wq_sb[:, ds(2 * kk, 2), ts(jt, P)],
                    xTq_sb[:, ds(2 * kk, 2), :],
                    start=(kk == 0), stop=(kk == NJ // 2 - 1),
                    perf_mode=DR)
            nc.vector.tensor_scalar(
                Q2_sb[:, jt, 0, :], ps[:],
                1.0 / 16.0, bq_sb[:, jt:jt + 1], ALU.mult, ALU.add)

        def ln_dve(out_ap, x_ap, sqd_ap, a_bc, g_bc):
            """out = LN(x) via DVE only (x preserved, sqd clobbered)."""
            st = late.tile([P, 8], F32, tag="lnstat", bufs=2)
            nc.vector.reduce_sum(st[:, 0:1], x_ap, axis=AX.X)
            nc.vector.tensor_scalar_mul(st[:, 1:2], st[:, 0:1], 1.0 / D)
            nc.vector.tensor_scalar_mul(st[:, 2:3], st[:, 0:1], -1.0 / D)
            nc.vector.affine_mul_reduce(
                sqd_ap, st[:, 3:4], x_ap, x_ap, 1.0, st[:, 2:3])
            # rstd = 1/sqrt(sum/D + eps)  (tiny Act op, [P,1])
            nc.scalar.activation(st[:, 4:5], st[:, 3:4], ACTF.Sqrt,
                                 scale=1.0 / D, bias=eps_bc)
            nc.vector.reciprocal(st[:, 5:6], st[:, 4:5])
            nc.vector.tensor_tensor(st[:, 6:7], st[:, 5:6], a_bc, ALU.mult)
            # g2 = gamma - mean*k ; out = x*k + g2
            nc.vector.tensor_tensor(st[:, 7:8], st[:, 1:2], st[:, 6:7],
                                    ALU.mult)
            nc.vector.tensor_tensor(st[:, 7:8], g_bc, st[:, 7:8],
                                    ALU.subtract)
            nc.vector.tensor_scalar(out_ap, x_ap, st[:, 6:7], st[:, 7:8],
                                    ALU.mult, ALU.add)

        def e_ln(tt):
            """O-proj + residual + LN1 for query tile tt."""
            res = late.tile([P, D], F32, tag="res1", name=f"res1_{tt}")
            ON = 512
            for no in range(D // ON):
                pso = pse.tile([P, ON], F32, tag="pe")
                for kk in range(NJ // 2):
                    nc.tensor.matmul(
                        pso[:], ctx_sb[:, ds(2 * kk, 2), ts(tt, P)],
                        wo_sb[:, ds(2 * kk, 2), ds(no * ON, ON)],
                        start=(kk == 0), stop=(kk == NJ // 2 - 1),
                        perf_mode=DR)
                nc.vector.tensor_tensor(
                    res[:, ds(no * ON, ON)], pso[:],
                    xqb_sb[:, tt, ds(no * ON, ON)], ALU.add)
            ln_dve(out1_sb[:, tt, :], res[:], sqd_sb[:],
                   alpha256_bc, gamma256_bc)

        def e_transpose(tt):
            """Transpose out1[tt] into fp8 hi/lo, then pre-add b2 residual."""
            for jt in range(NJ):
                pst = pse.tile([P, 1024], BF16, tag="pst")
                nc.tensor.transpose(
                    pst[:, 0:P], out1_sb[:, tt, ts(jt, P)], ident[:])
                mid = late.tile([P, P], BF16, tag="tmid", bufs=3)
                nc.vector.tensor_scalar_mul(mid[:], pst[:, 0:P], 1.0 / S_FF)
                nc.gpsimd.tensor_copy(out1T8[:, jt, ts(tt, P)], mid[:])
                nc.gpsimd.tensor_tensor(
                    out1T8l[:, jt, ts(tt, P)], mid[:],
                    out1T8[:, jt, ts(tt, P)], ALU.subtract)
            # out1b = 256*out1 + 256*b2 (FFN2 residual; transposes done)
            nc.gpsimd.tensor_tensor(
                out1_sb[:, tt, :], out1_sb[:, tt, :], b2s_bc[:], ALU.add)

        w1cache = {}

        def f1_chunk(mp, q0, qw):
            """FFN1 for w1 columns [512*mp, 512*mp+512), query cols [q0,q0+qw).
            Streams the w1 hi/lo chunk pair via fstream."""
            whi = fstream.tile([P, NJ, 512], FP8, tag="w1hic",
                               name=f"w1hi_{mp}_{q0}")
            nc.sync.dma_start(whi[:], w1hi_v[:, :, ds(mp * 512, 512)])
            wlo = fstream.tile([P, NJ, 512], FP8, tag="w1loc",
                               name=f"w1lo_{mp}_{q0}")
            nc.sync.dma_start(wlo[:], w1lo_v[:, :, ds(mp * 512, 512)])
            w1cache[mp] = (whi, wlo)
            f1_compute(whi, wlo, mp, q0, qw)

        def f1_compute(whi, wlo, mp, q0, qw):
            for mi in range(4):
                mt = mp * 4 + mi
                ps = pse.tile([P, TN], F32, tag="pe")
                groups = [(whi, out1T8), (wlo, out1T8), (whi, out1T8l)]
                for gi, (wg, xg) in enumerate(groups):
                    for kk in range(NJ // 2):
                        nc.tensor.matmul(
                            ps[:, 0:qw], wg[:, ds(2 * kk, 2), ts(mi, P)],
                            xg[:, ds(2 * kk, 2), ds(q0, qw)],
                            start=(gi == 0 and kk == 0),
                            stop=(gi == 2 and kk == NJ // 2 - 1),
                            perf_mode=DR)
                hmid = fstream.tile([P, TN], BF16, tag="hmid", bufs=4)
                nc.vector.tensor_scalar(
                    hmid[:, 0:qw], ps[:, 0:qw],
                    b1x16[:, mt:mt + 1], 0.0, ALU.add, ALU.max)
                nc.gpsimd.tensor_copy(hid_sb[:, mt, ds(q0, qw)],
                                      hmid[:, 0:qw])
                nc.gpsimd.tensor_tensor(
                    hid_lo[:, mt, ds(q0, qw)], hmid[:, 0:qw],
                    hid_sb[:, mt, ds(q0, qw)], ALU.subtract)

        # ---------------- attention ----------------
        ident = small.tile([P, P], BF16, tag="ident")
        from concourse.masks import make_identity
        make_identity(nc, ident)

        fillers = []

        def run_fillers(n):
            for _ in range(n):
                if fillers:
                    fillers.pop(0)()

        def attention_half(half):
            q0 = half * MQ
            for hg in range(H // HG):
                heads = range(hg * HG, (hg + 1) * HG)
                c2s = {h: psc.tile([P, 512], F32, tag="c2",
                       name=f"c2_{half}_{h}") for h in heads}
                exs = {}
                for q4 in range(NQ4 + 1):
                    if q4 < NQ4:
                        for h in heads:
                            hp = (h % HPJ) * DK
                            hj = h // HPJ
                            ps4 = psd.tile([P, 4, MQ], F32, tag="ps4")
                            for i in range(4):
                                mt = q4 * 4 + i
                                nc.tensor.matmul(
                                    ps4[:, i],
                                    K_sb[ds(hp, DK), hj,
                                         ds(mt * P, 2 * P)].rearrange(
                                        "p (u t) -> p u t", u=2),
                                    Q2_sb[ds(hp, DK), hj, :, ds(q0, MQ)],
                                    start=(i % 2 == 0), stop=True,
                                    perf_mode=DR, skip_group_check=True)
                            ex = dpool.tile([P, 4, MQ], FP8, tag="ex",
                                            bufs=5, name=f"ex{half}_{h}_{q4}")
                            nc.scalar.activation(
                                ex[:], ps4[:], ACTF.Exp,
                                scale=1.0 / math.sqrt(DK), bias=lnc_bc[:])
                            exs[(h, q4)] = ex
                    if q4 >= 1:
                        for h in heads:
                            ex = exs.pop((h, q4 - 1))
                            for j in range(2):
                                bp = (q4 - 1) * 2 + j
                                nc.tensor.matmul(
                                    c2s[h][0:DK + 1, 0:MQ],
                                    V_sb[:, ds(2 * bp, 2), h, :],
                                    ex[:, ds(2 * j, 2), :],
                                    start=(bp == 0), stop=(bp == NT // 2 - 1),
                                    perf_mode=DR)
                    run_fillers(2)
                for h in heads:
                    hp = (h % HPJ) * DK
                    hj = h // HPJ
                    recip = dpool.tile([1, MQ], BF16, tag="recip")
                    with nc.allow_low_precision(reason="fp8 ctx"):
                        nc.vector.reciprocal(recip[:],
                                             c2s[h][DK:DK + 1, 0:MQ])
                    recip_bc = dpool.tile([DK, MQ], BF16, tag="recip_bc")
                    nc.gpsimd.partition_broadcast(recip_bc[:], recip[:])
                    # ctx8 = c2/denom = 32*ctx exactly (scales cancel)
                    nc.vector.tensor_tensor(
                        ctx_sb[ds(hp, DK), hj, ds(q0, MQ)],
                        c2s[h][0:DK, 0:MQ], recip_bc[:], ALU.mult)
                run_fillers(2)

        # half A: fillers = V tiles (front-loaded for the attnV lag) + K/Q
        kq_proj(0)
        fillers = [lambda tt=tt: v_tiles(2 * tt, 2) for tt in range(NT // 2)]
        fillers += [lambda jt=jt: kq_proj(jt) for jt in range(1, NJ)]
        attention_half(0)
        while fillers:
            fillers.pop(0)()

        # release xt + projection weights (dead once K/Q/V are built)
        xtwp_cm.__exit__(None, None, None)

        # half B: fillers = E(half A) + FFN1(half A); LN1 and its dependent
        # transposes are separate items so PE work never queues behind a
        # DVE chain that hasn't drained yet
        fillers = [lambda: e_ln(0), lambda: e_ln(1),
                   lambda: e_transpose(0), lambda: e_transpose(1)]
        fillers += [lambda mp=mp: f1_chunk(mp, 0, MQ)
                    for mp in range(DFF // 512)]
        attention_half(1)
        while fillers:
            fillers.pop(0)()

        dp_cm.__exit__(None, None, None)
        kqv_cm.__exit__(None, None, None)    # release K, Q2, V

        # ---------------- post-B: E(B), FFN1(B), FFN2, LN2 ----------------
        post_cm = tc.tile_pool(name="post", bufs=1)
        post = post_cm.__enter__()

        ON = 512

        def w2_fetch(no):
            whi2 = post.tile([P, NF, ON], FP8, tag="w2hic", bufs=2,
                             name=f"w2hi_{no}")
            nc.sync.dma_start(whi2[:], w2hi_v[:, :, ds(no * ON, ON)])
            wlo2 = post.tile([P, NF, ON], FP8, tag="w2loc", bufs=2,
                             name=f"w2lo_{no}")
            nc.sync.dma_start(wlo2[:], w2lo_v[:, :, ds(no * ON, ON)])
            return whi2, wlo2

        w2q = {0: w2_fetch(0)}   # no0 weights stream during E-B/FFN1-B

        for tt in range(NTQ // 2, NTQ):
            e_ln(tt)
            e_transpose(tt)
        # FFN1 for half B: the last two w1 chunk pairs are still resident
        # in fstream's two buffers -> no re-DMA; the rest re-streams
        for mp in (7, 6):
            whi, wlo = w1cache[mp]
            f1_compute(whi, wlo, mp, MQ, MQ)
        for mp in reversed(range(DFF // 512 - 2)):
            f1_chunk(mp, MQ, MQ)

        # FFN2: w2 streamed once in output-column halves serving all tt;
        # res2 = 256*(hid@w2) + 256*(out1+b2); LN2 + store per tt
        w2q[1] = w2_fetch(1)

        for no in range(D // ON):
            whi2, wlo2 = w2q[no]
            for tt in range(NTQ):
                ps = pse.tile([P, ON], F32, tag="pe")
                groups = [(hid_sb, whi2), (hid_sb, wlo2), (hid_lo, whi2)]
                for gi, (hg_, wg) in enumerate(groups):
                    for kk in range(NF // 2):
                        nc.tensor.matmul(
                            ps[:], hg_[:, ds(2 * kk, 2), ts(tt, P)],
                            wg[:, ds(2 * kk, 2), :],
                            start=(gi == 0 and kk == 0),
                            stop=(gi == 2 and kk == NF // 2 - 1),
                            perf_mode=DR)
                nc.vector.tensor_tensor(
                    res2_sb[:, tt, ds(no * ON, ON)], ps[:],
                    out1_sb[:, tt, ds(no * ON, ON)], ALU.add)
                if no == D // ON - 1:
                    ln_dve(res2_sb[:, tt, :], res2_sb[:, tt, :], sqd_sb[:],
                           alpha_bc, gamma_bc)
                    nc.sync.dma_start(out_v[:, tt, :], res2_sb[:, tt, :])

        post_cm.__exit__(None, None, None)
        pse_cm.__exit__(None, None, None)
        psc_cm.__exit__(None, None, None)
        psd_cm.__exit__(None, None, None)
        fs_cm.__exit__(None, None, None)
        late_cm.__exit__(None, None, None)
        ctx2_cm.__exit__(None, None, None)

    return nc


_B, _S, _D, _H, _DK, _DFF = 2, 2048, 1024, 16, 64, 4096
_NCORES = 8
_TQ = (_B * _S) // _NCORES    # 512 query tokens per core

_cache = {}


def _get_program():
    if "nc" not in _cache:
        from concourse import bacc
        nc = bacc.Bacc("TRN2", target_bir_lowering=False, debug=False,
                       num_devices=_NCORES)
        build(nc, S=_S, D=_D, H=_H, DK=_DK, DFF=_DFF, TQ=_TQ)
        nc.compile()
        _cache["nc"] = nc
    return _cache["nc"]


def _core_inputs(inp):
    """Host-side prep: per-core input dicts (transposes + fp8 casts only)."""
    import ml_dtypes
    f8 = ml_dtypes.float8_e4m3

    def t8(a, s):
        return np.ascontiguousarray(
            np.asarray(a, np.float32).T * s).astype(f8)

    def hilo(a, s):
        t = np.ascontiguousarray(np.asarray(a, np.float32).T) * s
        hi = t.astype(f8)
        lo = (t - hi.astype(np.float32)).astype(f8)
        return hi, lo

    w1hi, w1lo = hilo(inp["w1"], 16.0)
    w2hi, w2lo = hilo(inp["w2"], 16.0)
    w = {
        "wq8": t8(inp["wq"], 16.0), "wk8": t8(inp["wk"], 16.0),
        "wv8": t8(inp["wv"], 32.0), "wo8": t8(inp["wo"], 32.0),
        "w1hi": w1hi, "w1lo": w1lo, "w2hi": w2hi, "w2lo": w2lo,
        "bq": np.asarray(inp["bq"]), "bk": np.asarray(inp["bk"]),
        "b1": np.asarray(inp["b1"]), "b2": np.asarray(inp["b2"]),
        "alpha": np.asarray(inp["alpha"]), "gamma": np.asarray(inp["gamma"]),
    }
    x = np.asarray(inp["x"], np.float32)
    # bv folded through the O-projection: ctx uses bias-free v, and
    # sum(attn)=1 makes the correction an additive constant bv @ wo^T
    bo = (np.asarray(inp["bo"], np.float32)
          + np.asarray(inp["bv"], np.float32)
          @ np.asarray(inp["wo"], np.float32).T)
    per_batch = _NCORES // _B
    maps = []
    for c in range(_NCORES):
        b, q0 = c // per_batch, (c % per_batch) * _TQ
        xb = x[b]
        m = dict(w)
        m["xT8"] = np.ascontiguousarray(xb.T).astype(f8)
        m["xTq8"] = np.ascontiguousarray(xb[q0:q0 + _TQ].T).astype(f8)
        m["xqb"] = np.ascontiguousarray(
            (xb[q0:q0 + _TQ] + bo) * S_RES).astype(ml_dtypes.bfloat16)
        maps.append(m)
    return maps


def kernel(**inputs) -> np.ndarray:
    from concourse.bass_utils import run_bass_kernel_spmd
    nc = _get_program()
    in_maps = _core_inputs(inputs)
    res = run_bass_kernel_spmd(nc, in_maps, core_ids=list(range(_NCORES)))
    out = np.empty((_B, _S, _D), dtype=np.float32)
    per_batch = _NCORES // _B
    for c, rm in enumerate(res.results):
        b, q0 = c // per_batch, (c % per_batch) * _TQ
        out[b, q0:q0 + _TQ] = rm["out"]
    return out


# revision 84
# speedup vs baseline: 1.0694x; 1.0694x over previous
"""Self-contained Trainium2 Bass kernel for the nn_EnocoderBlock problem.

kernel(**inputs) takes the full (unsharded) inputs of the reference encoder
block (B=2, S=2048, D=1024, H=16, DFF=4096) and returns the full [B, S, D]
fp32 output, running SPMD on 8 NeuronCores.

Sharding: data-parallel over batch x query-token blocks — each of the 8
cores owns one batch element's full K/V context and a 512-token query
slice, so no cross-core collectives are needed.

Precision: all large GEMMs run in fp8e4m3 with DoubleRow perf mode (0.5
PE cycles per output column).  The QK^T scores (64-deep contraction) use
DoubleRow with a zero second slot in the moving operand.  FFN weights
and activations are split into fp8 hi+lo pairs (error compensation), so
the end-to-end error stays ~2e-3.  Scale factors fold into weights / the
exp bias / LayerNorm constants (LayerNorm is scale-invariant).

Schedule: softmax exp on the Activation engine is the critical resource
(~133us).  Attention runs in two query halves; the exp stream starts as
early as possible and everything else (V/K/Q projections in half A; the
O-projection, LayerNorm1, transposes and FFN1 of half A inside half B's
window) is interleaved into the exp-bound windows as PE/DVE/Pool filler.
LayerNorms run DVE-only (affine_mul_reduce) to keep Act pure-exp.
"""

import sys
for _p in ("/opt/trn_rl_repo", "/root/.axon_site/_ro/trn_rl_repo"):
    if _p not in sys.path:
        sys.path.append(_p)

import numpy as np

import math
from contextlib import ExitStack

import concourse.mybir as mybir
import concourse.tile as tile
from concourse.bass import ds, ts

F32 = mybir.dt.float32
BF16 = mybir.dt.bfloat16
FP8 = mybir.dt.float8e4
AX = mybir.AxisListType
ALU = mybir.AluOpType
ACTF = mybir.ActivationFunctionType
DR = mybir.MatmulPerfMode.DoubleRow

P = 128
EPS = 1e-6
LNC = math.log(4.0)       # exp scale constant folded into activation bias
S_RES = 1024.0            # attention residual pre-scale (ctx32 @ wo32)
S_FF = 256.0              # ffn residual pre-scale (hid16 @ w216)


def build(nc, S=2048, D=1024, H=16, DK=64, DFF=4096, TQ=512):
    assert DK == 64 and D % P == 0 and S % P == 0 and DFF % P == 0
    NJ = D // P            # feature tiles of 128 (8)
    NT = S // P            # token tiles of 128 (16)
    NTQ = TQ // P          # query token tiles of 128 (4)
    NF = DFF // P          # dff tiles of 128 (32)
    HPJ = P // DK          # heads per 128-feature tile (2)
    HG = 2                 # attention head-group size
    TN = 512               # moving-dim tile (tokens)
    NTN = S // TN          # 4
    MQ = TQ // 2           # query-half width (256)
    NQ4 = NT // 4          # score quads per head per half (4)

    # ---------------- DRAM I/O ----------------
    def din(name, shape, dt):
        return nc.dram_tensor(name, shape, dt, kind="ExternalInput").ap()

    xT8 = din("xT8", [D, S], FP8)
    xTq8 = din("xTq8", [D, TQ], FP8)
    xqb = din("xqb", [TQ, D], BF16)           # S_RES * (x_q + bo + bv@wo^T)
    wv8, wk8 = din("wv8", [D, D], FP8), din("wk8", [D, D], FP8)
    wq8, wo8 = din("wq8", [D, D], FP8), din("wo8", [D, D], FP8)
    w1hi, w1lo = din("w1hi", [D, DFF], FP8), din("w1lo", [D, DFF], FP8)
    w2hi, w2lo = din("w2hi", [DFF, D], FP8), din("w2lo", [DFF, D], FP8)
    bq, bk = din("bq", [D], F32), din("bk", [D], F32)
    b1, b2 = din("b1", [DFF], F32), din("b2", [D], F32)
    alpha, gamma = din("alpha", [1], F32), din("gamma", [1], F32)
    out = nc.dram_tensor("out", [TQ, D], F32, kind="ExternalOutput").ap()

    xT_v = xT8.rearrange("(o p) t -> p o t", p=P)
    xTq_v = xTq8.rearrange("(o p) t -> p o t", p=P)
    xqb_v = xqb.rearrange("(o p) d -> p o d", p=P)
    out_v = out.rearrange("(o p) d -> p o d", p=P)
    wv_v = wv8.rearrange("(o p) j -> p o j", p=P)
    wk_v = wk8.rearrange("(o p) j -> p o j", p=P)
    wq_v = wq8.rearrange("(o p) j -> p o j", p=P)
    wo_v = wo8.rearrange("(o p) j -> p o j", p=P)
    w1hi_v = w1hi.rearrange("(o p) f -> p o f", p=P)
    w1lo_v = w1lo.rearrange("(o p) f -> p o f", p=P)
    w2hi_v = w2hi.rearrange("(o p) j -> p o j", p=P)
    w2lo_v = w2lo.rearrange("(o p) j -> p o j", p=P)
    bq_v = bq.rearrange("(o p) -> p o", p=P)
    bk_v = bk.rearrange("(o p) -> p o", p=P)
    b1_v = b1.rearrange("(o p) -> p o", p=P)

    with tile.TileContext(nc) as tc, ExitStack() as octx:
        small = octx.enter_context(tc.tile_pool(name="small", bufs=1))

        # ============ pools (LIFO; xtwp dies mid-A, kqv at B-end) ========
        ctx2_cm = tc.tile_pool(name="ctx2", bufs=1)
        ctx2 = ctx2_cm.__enter__()
        ctx_sb = ctx2.tile([P, NJ, TQ], FP8, tag="ctx")
        xqb_sb = ctx2.tile([P, NTQ, D], BF16, tag="xqb")

        late_cm = tc.tile_pool(name="late", bufs=1)
        late = late_cm.__enter__()
        out1_sb = late.tile([P, NTQ, D], F32, tag="out1")    # 256*out1
        out1T8 = late.tile([P, NJ, TQ], FP8, tag="out1T")
        out1T8l = late.tile([P, NJ, TQ], FP8, tag="out1Tl")
        hid_sb = late.tile([P, NF, TQ], FP8, tag="hid")      # 16*relu hi
        hid_lo = late.tile([P, NF, TQ], FP8, tag="hidlo")
        res2_sb = late.tile([P, NTQ, D], BF16, tag="res2")

        fs_cm = tc.tile_pool(name="fstream", bufs=2)
        fstream = fs_cm.__enter__()

        kqv_cm = tc.tile_pool(name="kqv", bufs=1)
        kqv = kqv_cm.__enter__()
        # K has a zeroed 128-token tail: the DR scores lhsT uses 2 token-
        # tile slots and slot 1 (multiplying Q2's zero slot) must be finite
        K_sb = kqv.tile([P, NJ, S + P], FP8, tag="K")
        Q2_sb = kqv.tile([P, NJ, 2, TQ], FP8, tag="Q2")      # [Q; 0] pairs
        V_sb = kqv.tile([P, NT, H, DK + 1], FP8, tag="V")

        dp_cm = tc.tile_pool(name="dpool", bufs=2)
        dpool = dp_cm.__enter__()

        xtwp_cm = tc.tile_pool(name="xtwp", bufs=1)
        xtwp = xtwp_cm.__enter__()
        xt_all = xtwp.tile([P, NJ, S], FP8, tag="xt")
        wv_sb = xtwp.tile([P, NJ, D], FP8, tag="wv")
        wk_sb = xtwp.tile([P, NJ, D], FP8, tag="wk")
        wq_sb = xtwp.tile([P, NJ, D], FP8, tag="wq")
        xTq_sb = xtwp.tile([P, NJ, TQ], FP8, tag="xTq")

        # ---- input DMAs, ordered for streaming ----
        nc.sync.dma_start(wv_sb[:], wv_v)
        bq_sb = small.tile([P, NJ], F32, tag="bq")
        bk_sb = small.tile([P, NJ], F32, tag="bk")
        XC = 512
        for c in range(S // XC):
            nc.sync.dma_start(xt_all[:, :, ds(c * XC, XC)],
                              xT_v[:, :, ds(c * XC, XC)])
            if c == 1:
                nc.sync.dma_start(wk_sb[:], wk_v)
        nc.sync.dma_start(bk_sb[:], bk_v)
        nc.sync.dma_start(bq_sb[:], bq_v)
        nc.sync.dma_start(wq_sb[:], wq_v)
        nc.sync.dma_start(xTq_sb[:], xTq_v)
        nc.gpsimd.memset(Q2_sb[:, :, 1, :], 0.0)
        nc.gpsimd.memset(K_sb[:, :, S:], 0.0)
        nc.vector.memset(V_sb[:, :, :, DK:DK + 1], 1.0)

        nc.sync.dma_start(xqb_sb[:], xqb_v)

        # ---------------- constants / biases ----------------
        b1_sb = small.tile([P, NF], F32, tag="b1")
        nc.sync.dma_start(b1_sb[:], b1_v)
        b1x16 = small.tile([P, NF], F32, tag="b1x16")
        nc.vector.tensor_scalar_mul(b1x16[:], b1_sb[:], 16.0)

        # row staging goes through partition 0 of the (not-yet-used) LN
        # squares-dump tile — SBUF is too tight for a dedicated rows pool
        sqd_sb = late.tile([P, D], F32, tag="sqd")
        nc.sync.dma_start(sqd_sb[0:1, :], b2[None, :])
        b2s_bc = small.tile([P, D], BF16, tag="b2s_bc")
        nc.vector.tensor_scalar_mul(b2s_bc[0:1, :], sqd_sb[0:1, :], S_FF)
        nc.gpsimd.partition_broadcast(b2s_bc[:], b2s_bc[0:1, :])

        ag_st = late.tile([P, 8], F32, tag="lnstat", bufs=2)
        nc.sync.dma_start(ag_st[0:1, 0:1], alpha[None, :])
        nc.sync.dma_start(ag_st[0:1, 1:2], gamma[None, :])
        ag_bc = small.tile([P, 2], F32, tag="ag_bc")
        nc.gpsimd.partition_broadcast(ag_bc[:], ag_st[0:1, 0:2])
        alpha_bc = ag_bc[:, 0:1]
        gamma_bc = ag_bc[:, 1:2]
        ag256 = small.tile([P, 2], F32, tag="ag256")
        nc.vector.tensor_scalar_mul(ag256[:], ag_bc[:], S_FF)
        alpha256_bc = ag256[:, 0:1]
        gamma256_bc = ag256[:, 1:2]

        eps_bc = small.tile([P, 1], F32, tag="eps_bc")
        nc.vector.memset(eps_bc[:], EPS)
        lnc_bc = small.tile([P, 1], F32, tag="lnc_bc")
        nc.vector.memset(lnc_bc[:], LNC)

        # ================= PSUM pools (8 banks total) =================
        psd_cm = tc.tile_pool(name="psd", bufs=2, space="PSUM")
        psd = psd_cm.__enter__()        # ps4 [P,4,MQ] f32 = 2 banks x2
        psc_cm = tc.tile_pool(name="psc", bufs=2, space="PSUM")
        psc = psc_cm.__enter__()        # c2 [P,512] f32 = 1 bank x2
        pse_cm = tc.tile_pool(name="pse", bufs=2, space="PSUM")
        pse = pse_cm.__enter__()        # pe [P,512] f32 x2 (fillers)

        # ---------------- filler helpers ----------------
        def v_tiles(tt0, n):
            """V projection for token tiles tt0..tt0+n-1 (32*v, no bias)."""
            VN = 512
            for tt in range(tt0, tt0 + n):
                for nv in range(D // VN):
                    ps = pse.tile([P, VN], F32, tag="pe")
                    for kk in range(NJ // 2):
                        nc.tensor.matmul(
                            ps[:], xt_all[:, ds(2 * kk, 2), ts(tt, P)],
                            wv_sb[:, ds(2 * kk, 2), ds(nv * VN, VN)],
                            start=(kk == 0), stop=(kk == NJ // 2 - 1),
                            perf_mode=DR)
                    # alternate drain engines so neither DVE nor Act
                    # becomes the V-phase bottleneck
                    if nv == 0:
                        nc.vector.tensor_copy(
                            V_sb[:, tt, ds(0, VN // DK), 0:DK],
                            ps[:].rearrange("p (h d) -> p h d", d=DK))
                    else:
                        nc.scalar.activation(
                            V_sb[:, tt, ds(VN // DK, VN // DK), 0:DK],
                            ps[:].rearrange("p (h d) -> p h d", d=DK),
                            ACTF.Identity)

        def kq_proj(jt):
            for nt in range(NTN):
                ps = pse.tile([P, TN], F32, tag="pe")
                for kk in range(NJ // 2):
                    nc.tensor.matmul(
                        ps[:], wk_sb[:, ds(2 * kk, 2), ts(jt, P)],
                        xt_all[:, ds(2 * kk, 2), ds(nt * TN, TN)],
                        start=(kk == 0), stop=(kk == NJ // 2 - 1),
                        perf_mode=DR)
                nc.vector.tensor_scalar(
                    K_sb[:, jt, ds(nt * TN, TN)], ps[:],
                    1.0 / 16.0, bk_sb[:, jt:jt + 1], ALU.mult, ALU.add)
            ps = pse.tile([P, TQ], F32, tag="pe")
            for kk in range(NJ // 2):
                nc.tensor.matmul(
                    ps[:], wq_sb[:, ds(2 * kk, 2), ts(jt, P)],
                    xTq_sb[:, ds(2 * kk, 2), :],
                    start=(kk == 0), stop=(kk == NJ // 2 - 1),
                    perf_mode=DR)
            nc.vector.tensor_scalar(
                Q2_sb[:, jt, 0, :], ps[:],
                1.0 / 16.0, bq_sb[:, jt:jt + 1], ALU.mult, ALU.add)

        def ln_dve(out_ap, x_ap, sqd_ap, a_bc, g_bc):
            """out = LN(x) via DVE only (x preserved, sqd clobbered)."""
            st = late.tile([P, 8], F32, tag="lnstat", bufs=2)
            nc.vector.reduce_sum(st[:, 0:1], x_ap, axis=AX.X)
            nc.vector.tensor_scalar_mul(st[:, 1:2], st[:, 0:1], 1.0 / D)
            nc.vector.tensor_scalar_mul(st[:, 2:3], st[:, 0:1], -1.0 / D)
            nc.vector.affine_mul_reduce(
                sqd_ap, st[:, 3:4], x_ap, x_ap, 1.0, st[:, 2:3])
            # rstd = 1/sqrt(sum/D + eps)  (tiny Act op, [P,1])
            nc.scalar.activation(st[:, 4:5], st[:, 3:4], ACTF.Sqrt,
                                 scale=1.0 / D, bias=eps_bc)
            nc.vector.reciprocal(st[:, 5:6], st[:, 4:5])
            nc.vector.tensor_tensor(st[:, 6:7], st[:, 5:6], a_bc, ALU.mult)
            # g2 = gamma - mean*k ; out = x*k + g2
            nc.vector.tensor_tensor(st[:, 7:8], st[:, 1:2], st[:, 6:7],
                                    ALU.mult)
            nc.vector.tensor_tensor(st[:, 7:8], g_bc, st[:, 7:8],
                                    ALU.subtract)
            nc.vector.tensor_scalar(out_ap, x_ap, st[:, 6:7], st[:, 7:8],
                                    ALU.mult, ALU.add)

        def e_lns(tts):
            """O-proj + residual + LN1 for query tiles tts (wo streamed)."""
            ON = 512
            reses = {tt: late.tile([P, D], F32, tag="res1", bufs=2,
                                   name=f"res1_{tt}") for tt in tts}
            for no in range(D // ON):
                woc = fstream.tile([P, NJ, ON], FP8, tag="woc", bufs=1,
                                   name=f"wo_{tts[0]}_{no}")
                nc.sync.dma_start(woc[:], wo_v[:, :, ds(no * ON, ON)])
                for tt in tts:
                    pso = pse.tile([P, ON], F32, tag="pe")
                    for kk in range(NJ // 2):
                        nc.tensor.matmul(
                            pso[:], ctx_sb[:, ds(2 * kk, 2), ts(tt, P)],
                            woc[:, ds(2 * kk, 2), :],
                            start=(kk == 0), stop=(kk == NJ // 2 - 1),
                            perf_mode=DR)
                    nc.vector.tensor_tensor(
                        reses[tt][:, ds(no * ON, ON)], pso[:],
                        xqb_sb[:, tt, ds(no * ON, ON)], ALU.add)
            for tt in tts:
                ln_dve(out1_sb[:, tt, :], reses[tt][:], sqd_sb[:],
                       alpha256_bc, gamma256_bc)

        def e_transpose(tt):
            """Transpose out1[tt] into fp8 hi/lo, then pre-add b2 residual."""
            for jt in range(NJ):
                pst = pse.tile([P, 512], F32, tag="pe")
                nc.tensor.transpose(
                    pst[:, 0:P], out1_sb[:, tt, ts(jt, P)], ident[:])
                mid = late.tile([P, P], BF16, tag="tmid", bufs=2)
                nc.vector.tensor_scalar_mul(mid[:], pst[:, 0:P], 1.0 / S_FF)
                nc.gpsimd.tensor_copy(out1T8[:, jt, ts(tt, P)], mid[:])
                nc.gpsimd.tensor_tensor(
                    out1T8l[:, jt, ts(tt, P)], mid[:],
                    out1T8[:, jt, ts(tt, P)], ALU.subtract)
            # out1b = 256*out1 + 256*b2 (FFN2 residual; transposes done)
            nc.gpsimd.tensor_tensor(
                out1_sb[:, tt, :], out1_sb[:, tt, :], b2s_bc[:], ALU.add)

        w1cache = {}

        def f1_chunk(mp, q0, qw):
            """FFN1 for w1 columns [512*mp, 512*mp+512), query cols [q0,q0+qw).
            Streams the w1 hi/lo chunk pair via fstream."""
            whi = fstream.tile([P, NJ, 512], FP8, tag="w1hic",
                               name=f"w1hi_{mp}_{q0}")
            nc.sync.dma_start(whi[:], w1hi_v[:, :, ds(mp * 512, 512)])
            wlo = fstream.tile([P, NJ, 512], FP8, tag="w1loc",
                               name=f"w1lo_{mp}_{q0}")
            nc.sync.dma_start(wlo[:], w1lo_v[:, :, ds(mp * 512, 512)])
            w1cache[mp] = (whi, wlo)
            f1_compute(whi, wlo, mp, q0, qw)

        def f1_compute(whi, wlo, mp, q0, qw):
            for mi in range(4):
                mt = mp * 4 + mi
                ps = pse.tile([P, TN], F32, tag="pe")
                groups = [(whi, out1T8), (wlo, out1T8), (whi, out1T8l)]
                for gi, (wg, xg) in enumerate(groups):
                    for kk in range(NJ // 2):
                        nc.tensor.matmul(
                            ps[:, 0:qw], wg[:, ds(2 * kk, 2), ts(mi, P)],
                            xg[:, ds(2 * kk, 2), ds(q0, qw)],
                            start=(gi == 0 and kk == 0),
                            stop=(gi == 2 and kk == NJ // 2 - 1),
                            perf_mode=DR)
                hmid = fstream.tile([P, TN], BF16, tag="hmid", bufs=2)
                nc.vector.tensor_scalar(
                    hmid[:, 0:qw], ps[:, 0:qw],
                    b1x16[:, mt:mt + 1], 0.0, ALU.add, ALU.max)
                nc.gpsimd.tensor_copy(hid_sb[:, mt, ds(q0, qw)],
                                      hmid[:, 0:qw])
                nc.gpsimd.tensor_tensor(
                    hid_lo[:, mt, ds(q0, qw)], hmid[:, 0:qw],
                    hid_sb[:, mt, ds(q0, qw)], ALU.subtract)

        # ---------------- attention ----------------
        ident = small.tile([P, P], F32, tag="ident")
        from concourse.masks import make_identity
        make_identity(nc, ident)

        fillers = []

        def run_fillers(n):
            for _ in range(n):
                if fillers:
                    fillers.pop(0)()

        def attention_half(half):
            q0 = half * MQ
            for hg in range(H // HG):
                heads = range(hg * HG, (hg + 1) * HG)
                c2s = {h: psc.tile([P, 512], F32, tag="c2",
                       name=f"c2_{half}_{h}") for h in heads}
                exs = {}
                for q4 in range(NQ4 + 1):
                    if q4 < NQ4:
                        for h in heads:
                            hp = (h % HPJ) * DK
                            hj = h // HPJ
                            ps4 = psd.tile([P, 4, MQ], F32, tag="ps4")
                            for i in range(4):
                                mt = q4 * 4 + i
                                nc.tensor.matmul(
                                    ps4[:, i],
                                    K_sb[ds(hp, DK), hj,
                                         ds(mt * P, 2 * P)].rearrange(
                                        "p (u t) -> p u t", u=2),
                                    Q2_sb[ds(hp, DK), hj, :, ds(q0, MQ)],
                                    start=(i % 2 == 0), stop=True,
                                    perf_mode=DR, skip_group_check=True)
                            ex = dpool.tile([P, 4, MQ], FP8, tag="ex",
                                            bufs=4, name=f"ex{half}_{h}_{q4}")
                            nc.scalar.activation(
                                ex[:], ps4[:], ACTF.Exp,
                                scale=1.0 / math.sqrt(DK), bias=lnc_bc[:])
                            exs[(h, q4)] = ex
                    if q4 >= 1:
                        for h in heads:
                            ex = exs.pop((h, q4 - 1))
                            for j in range(2):
                                bp = (q4 - 1) * 2 + j
                                nc.tensor.matmul(
                                    c2s[h][0:DK + 1, 0:MQ],
                                    V_sb[:, ds(2 * bp, 2), h, :],
                                    ex[:, ds(2 * j, 2), :],
                                    start=(bp == 0), stop=(bp == NT // 2 - 1),
                                    perf_mode=DR)
                    run_fillers(2)
                for h in heads:
                    hp = (h % HPJ) * DK
                    hj = h // HPJ
                    recip = dpool.tile([1, MQ], BF16, tag="recip")
                    with nc.allow_low_precision(reason="fp8 ctx"):
                        nc.vector.reciprocal(recip[:],
                                             c2s[h][DK:DK + 1, 0:MQ])
                    recip_bc = dpool.tile([DK, MQ], BF16, tag="recip_bc")
                    nc.gpsimd.partition_broadcast(recip_bc[:], recip[:])
                    # ctx8 = c2/denom = 32*ctx exactly (scales cancel)
                    nc.vector.tensor_tensor(
                        ctx_sb[ds(hp, DK), hj, ds(q0, MQ)],
                        c2s[h][0:DK, 0:MQ], recip_bc[:], ALU.mult)
                run_fillers(2)

        # half A: fillers = V tiles (front-loaded for the attnV lag) + K/Q
        v_tiles(0, 4)      # before kq0: V streams as soon as wv+xt arrive
        kq_proj(0)
        fillers = [lambda tt=tt: v_tiles(2 * tt, 2) for tt in range(2, NT // 2)]
        fillers += [lambda jt=jt: kq_proj(jt) for jt in range(1, NJ)]
        attention_half(0)
        while fillers:
            fillers.pop(0)()

        # release xt + projection weights (dead once K/Q/V are built)
        xtwp_cm.__exit__(None, None, None)

        # half B: fillers = E(half A) + FFN1(half A); LN1 and its dependent
        # transposes are separate items so PE work never queues behind a
        # DVE chain that hasn't drained yet
        fillers = [lambda: e_lns([0, 1]),
                   lambda: e_transpose(0), lambda: e_transpose(1)]
        fillers += [lambda mp=mp: f1_chunk(mp, 0, MQ)
                    for mp in range(DFF // 512)]
        attention_half(1)
        while fillers:
            fillers.pop(0)()

        dp_cm.__exit__(None, None, None)
        kqv_cm.__exit__(None, None, None)    # release K, Q2, V

        # ---------------- post-B: E(B), FFN1(B), FFN2, LN2 ----------------
        post_cm = tc.tile_pool(name="post", bufs=1)
        post = post_cm.__enter__()

        ON = 512

        def w2_fetch(no):
            whi2 = post.tile([P, NF, ON], FP8, tag="w2hic", bufs=2,
                             name=f"w2hi_{no}")
            nc.sync.dma_start(whi2[:], w2hi_v[:, :, ds(no * ON, ON)])
            wlo2 = post.tile([P, NF, ON], FP8, tag="w2loc", bufs=2,
                             name=f"w2lo_{no}")
            nc.sync.dma_start(wlo2[:], w2lo_v[:, :, ds(no * ON, ON)])
            return whi2, wlo2

        w2q = {0: w2_fetch(0)}   # no0 weights stream during E-B/FFN1-B

        e_lns([2, 3])
        for tt in range(NTQ // 2, NTQ):
            e_transpose(tt)
        # FFN1 for half B: the last two w1 chunk pairs are still resident
        # in fstream's two buffers -> no re-DMA; the rest re-streams
        for mp in (7, 6):
            whi, wlo = w1cache[mp]
            f1_compute(whi, wlo, mp, MQ, MQ)
        for mp in reversed(range(DFF // 512 - 2)):
            f1_chunk(mp, MQ, MQ)

        # FFN2: w2 streamed once in output-column halves serving all tt;
        # res2 = 256*(hid@w2) + 256*(out1+b2); LN2 + store per tt
        w2q[1] = w2_fetch(1)

        for no in range(D // ON):
            whi2, wlo2 = w2q[no]
            for tt in range(NTQ):
                ps = pse.tile([P, ON], F32, tag="pe")
                groups = [(hid_sb, whi2), (hid_sb, wlo2), (hid_lo, whi2)]
                for gi, (hg_, wg) in enumerate(groups):
                    for kk in range(NF // 2):
                        nc.tensor.matmul(
                            ps[:], hg_[:, ds(2 * kk, 2), ts(tt, P)],
                            wg[:, ds(2 * kk, 2), :],
                            start=(gi == 0 and kk == 0),
                            stop=(gi == 2 and kk == NF // 2 - 1),
                            perf_mode=DR)
                nc.vector.tensor_tensor(
                    res2_sb[:, tt, ds(no * ON, ON)], ps[:],
                    out1_sb[:, tt, ds(no * ON, ON)], ALU.add)
                if no == D // ON - 1:
                    o2 = late.tile([P, D], F32, tag="res1", bufs=2,
                                   name=f"o2_{tt}")
                    ln_dve(o2[:], res2_sb[:, tt, :], sqd_sb[:],
                           alpha_bc, gamma_bc)
                    nc.sync.dma_start(out_v[:, tt, :], o2[:])

        post_cm.__exit__(None, None, None)
        pse_cm.__exit__(None, None, None)
        psc_cm.__exit__(None, None, None)
        psd_cm.__exit__(None, None, None)
        fs_cm.__exit__(None, None, None)
        late_cm.__exit__(None, None, None)
        ctx2_cm.__exit__(None, None, None)

    return nc


_B, _S, _D, _H, _DK, _DFF = 2, 2048, 1024, 16, 64, 4096
_NCORES = 8
_TQ = (_B * _S) // _NCORES    # 512 query tokens per core

_cache = {}


def _get_program():
    if "nc" not in _cache:
        from concourse import bacc
        nc = bacc.Bacc("TRN2", target_bir_lowering=False, debug=False,
                       num_devices=_NCORES)
        build(nc, S=_S, D=_D, H=_H, DK=_DK, DFF=_DFF, TQ=_TQ)
        nc.compile()
        _cache["nc"] = nc
    return _cache["nc"]


def _core_inputs(inp):
    """Host-side prep: per-core input dicts (transposes + fp8 casts only)."""
    import ml_dtypes
    f8 = ml_dtypes.float8_e4m3

    def t8(a, s):
        return np.ascontiguousarray(
            np.asarray(a, np.float32).T * s).astype(f8)

    def hilo(a, s):
        t = np.ascontiguousarray(np.asarray(a, np.float32).T) * s
        hi = t.astype(f8)
        lo = (t - hi.astype(np.float32)).astype(f8)
        return hi, lo

    w1hi, w1lo = hilo(inp["w1"], 16.0)
    w2hi, w2lo = hilo(inp["w2"], 16.0)
    w = {
        "wq8": t8(inp["wq"], 16.0), "wk8": t8(inp["wk"], 16.0),
        "wv8": t8(inp["wv"], 32.0), "wo8": t8(inp["wo"], 32.0),
        "w1hi": w1hi, "w1lo": w1lo, "w2hi": w2hi, "w2lo": w2lo,
        "bq": np.asarray(inp["bq"]), "bk": np.asarray(inp["bk"]),
        "b1": np.asarray(inp["b1"]), "b2": np.asarray(inp["b2"]),
        "alpha": np.asarray(inp["alpha"]), "gamma": np.asarray(inp["gamma"]),
    }
    x = np.asarray(inp["x"], np.float32)
    # bv folded through the O-projection: ctx uses bias-free v, and
    # sum(attn)=1 makes the correction an additive constant bv @ wo^T
    bo = (np.asarray(inp["bo"], np.float32)
          + np.asarray(inp["bv"], np.float32)
          @ np.asarray(inp["wo"], np.float32).T)
    per_batch = _NCORES // _B
    maps = []
    for c in range(_NCORES):
        b, q0 = c // per_batch, (c % per_batch) * _TQ
        xb = x[b]
        m = dict(w)
        m["xT8"] = np.ascontiguousarray(xb.T).astype(f8)
        m["xTq8"] = np.ascontiguousarray(xb[q0:q0 + _TQ].T).astype(f8)
        m["xqb"] = np.ascontiguousarray(
            (xb[q0:q0 + _TQ] + bo) * S_RES).astype(ml_dtypes.bfloat16)
        maps.append(m)
    return maps


def kernel(**inputs) -> np.ndarray:
    from concourse.bass_utils import run_bass_kernel_spmd
    nc = _get_program()
    in_maps = _core_inputs(inputs)
    res = run_bass_kernel_spmd(nc, in_maps, core_ids=list(range(_NCORES)))
    out = np.empty((_B, _S, _D), dtype=np.float32)
    per_batch = _NCORES // _B
    for c, rm in enumerate(res.results):
        b, q0 = c // per_batch, (c % per_batch) * _TQ
        out[b, q0:q0 + _TQ] = rm["out"]
    return out


# revision 86
# speedup vs baseline: 1.0805x; 1.0104x over previous
"""Self-contained Trainium2 Bass kernel for the nn_EnocoderBlock problem.

kernel(**inputs) takes the full (unsharded) inputs of the reference encoder
block (B=2, S=2048, D=1024, H=16, DFF=4096) and returns the full [B, S, D]
fp32 output, running SPMD on 8 NeuronCores.

Sharding: data-parallel over batch x query-token blocks — each of the 8
cores owns one batch element's full K/V context and a 512-token query
slice, so no cross-core collectives are needed.

Precision: all large GEMMs run in fp8e4m3 with DoubleRow perf mode (0.5
PE cycles per output column).  The QK^T scores (64-deep contraction) use
DoubleRow with a zero second slot in the moving operand.  FFN weights
and activations are split into fp8 hi+lo pairs (error compensation), so
the end-to-end error stays ~2e-3.  Scale factors fold into weights / the
exp bias / LayerNorm constants (LayerNorm is scale-invariant).

Schedule: softmax exp on the Activation engine is the critical resource
(~133us).  Attention runs in two query halves; the exp stream starts as
early as possible and everything else (V/K/Q projections in half A; the
O-projection, LayerNorm1, transposes and FFN1 of half A inside half B's
window) is interleaved into the exp-bound windows as PE/DVE/Pool filler.
LayerNorms run DVE-only (affine_mul_reduce) to keep Act pure-exp.
"""

import sys
for _p in ("/opt/trn_rl_repo", "/root/.axon_site/_ro/trn_rl_repo"):
    if _p not in sys.path:
        sys.path.append(_p)

import numpy as np

import math
from contextlib import ExitStack

import concourse.mybir as mybir
import concourse.tile as tile
from concourse.bass import ds, ts

F32 = mybir.dt.float32
BF16 = mybir.dt.bfloat16
FP8 = mybir.dt.float8e4
AX = mybir.AxisListType
ALU = mybir.AluOpType
ACTF = mybir.ActivationFunctionType
DR = mybir.MatmulPerfMode.DoubleRow

P = 128
EPS = 1e-6
LNC = math.log(4.0)       # exp scale constant folded into activation bias
S_RES = 1024.0            # attention residual pre-scale (ctx32 @ wo32)
S_FF = 256.0              # ffn residual pre-scale (hid16 @ w216)


def build(nc, S=2048, D=1024, H=16, DK=64, DFF=4096, TQ=512):
    assert DK == 64 and D % P == 0 and S % P == 0 and DFF % P == 0
    NJ = D // P            # feature tiles of 128 (8)
    NT = S // P            # token tiles of 128 (16)
    NTQ = TQ // P          # query token tiles of 128 (4)
    NF = DFF // P          # dff tiles of 128 (32)
    HPJ = P // DK          # heads per 128-feature tile (2)
    HG = 2                 # attention head-group size
    TN = 512               # moving-dim tile (tokens)
    NTN = S // TN          # 4
    MQ = TQ // 2           # query-half width (256)
    NQ4 = NT // 4          # score quads per head per half (4)

    # ---------------- DRAM I/O ----------------
    def din(name, shape, dt):
        return nc.dram_tensor(name, shape, dt, kind="ExternalInput").ap()

    xT8 = din("xT8", [D, S], FP8)
    xTq8 = din("xTq8", [D, TQ], FP8)
    xqb = din("xqb", [TQ, D], BF16)           # S_RES * (x_q + bo + bv@wo^T)
    wv8, wk8 = din("wv8", [D, D], FP8), din("wk8", [D, D], FP8)
    wq8, wo8 = din("wq8", [D, D], FP8), din("wo8", [D, D], FP8)
    w1hi, w1lo = din("w1hi", [D, DFF], FP8), din("w1lo", [D, DFF], FP8)
    w2hi, w2lo = din("w2hi", [DFF, D], FP8), din("w2lo", [DFF, D], FP8)
    bq, bk = din("bq", [D], F32), din("bk", [D], F32)
    b1, b2 = din("b1", [DFF], F32), din("b2", [D], F32)
    alpha, gamma = din("alpha", [1], F32), din("gamma", [1], F32)
    out = nc.dram_tensor("out", [TQ, D], F32, kind="ExternalOutput").ap()

    xT_v = xT8.rearrange("(o p) t -> p o t", p=P)
    xTq_v = xTq8.rearrange("(o p) t -> p o t", p=P)
    xqb_v = xqb.rearrange("(o p) d -> p o d", p=P)
    out_v = out.rearrange("(o p) d -> p o d", p=P)
    wv_v = wv8.rearrange("(o p) j -> p o j", p=P)
    wk_v = wk8.rearrange("(o p) j -> p o j", p=P)
    wq_v = wq8.rearrange("(o p) j -> p o j", p=P)
    wo_v = wo8.rearrange("(o p) j -> p o j", p=P)
    w1hi_v = w1hi.rearrange("(o p) f -> p o f", p=P)
    w1lo_v = w1lo.rearrange("(o p) f -> p o f", p=P)
    w2hi_v = w2hi.rearrange("(o p) j -> p o j", p=P)
    w2lo_v = w2lo.rearrange("(o p) j -> p o j", p=P)
    bq_v = bq.rearrange("(o p) -> p o", p=P)
    bk_v = bk.rearrange("(o p) -> p o", p=P)
    b1_v = b1.rearrange("(o p) -> p o", p=P)

    with tile.TileContext(nc) as tc, ExitStack() as octx:
        small = octx.enter_context(tc.tile_pool(name="small", bufs=1))

        # ============ pools (LIFO; xtwp dies mid-A, kqv at B-end) ========
        ctx2_cm = tc.tile_pool(name="ctx2", bufs=1)
        ctx2 = ctx2_cm.__enter__()
        ctx_sb = ctx2.tile([P, NJ, TQ], FP8, tag="ctx")
        xqb_sb = ctx2.tile([P, NTQ, D], BF16, tag="xqb")

        late_cm = tc.tile_pool(name="late", bufs=1)
        late = late_cm.__enter__()
        out1_sb = late.tile([P, NTQ, D], F32, tag="out1")    # 256*out1
        out1T8 = late.tile([P, NJ, TQ], FP8, tag="out1T")
        out1T8l = late.tile([P, NJ, TQ], FP8, tag="out1Tl")
        hid_sb = late.tile([P, NF, TQ], FP8, tag="hid")      # 16*relu hi
        hid_lo = late.tile([P, NF, TQ], FP8, tag="hidlo")
        res2_sb = late.tile([P, NTQ, D], BF16, tag="res2")

        fs_cm = tc.tile_pool(name="fstream", bufs=2)
        fstream = fs_cm.__enter__()

        kqv_cm = tc.tile_pool(name="kqv", bufs=1)
        kqv = kqv_cm.__enter__()
        # K has a zeroed 128-token tail: the DR scores lhsT uses 2 token-
        # tile slots and slot 1 (multiplying Q2's zero slot) must be finite
        K_sb = kqv.tile([P, NJ, S + P], FP8, tag="K")
        Q2_sb = kqv.tile([P, NJ, 2, TQ], FP8, tag="Q2")      # [Q; 0] pairs
        V_sb = kqv.tile([P, NT, H, DK + 1], FP8, tag="V")

        dp_cm = tc.tile_pool(name="dpool", bufs=2)
        dpool = dp_cm.__enter__()

        xtwp_cm = tc.tile_pool(name="xtwp", bufs=1)
        xtwp = xtwp_cm.__enter__()
        xt_all = xtwp.tile([P, NJ, S], FP8, tag="xt")
        wv_sb = xtwp.tile([P, NJ, D], FP8, tag="wv")
        wk_sb = xtwp.tile([P, NJ, D], FP8, tag="wk")
        wq_sb = xtwp.tile([P, NJ, D], FP8, tag="wq")
        xTq_sb = xtwp.tile([P, NJ, TQ], FP8, tag="xTq")

        # ---- input DMAs, ordered for streaming ----
        nc.sync.dma_start(wv_sb[:], wv_v)
        bq_sb = small.tile([P, NJ], F32, tag="bq")
        bk_sb = small.tile([P, NJ], F32, tag="bk")
        XC = 512
        for c in range(S // XC):
            nc.sync.dma_start(xt_all[:, :, ds(c * XC, XC)],
                              xT_v[:, :, ds(c * XC, XC)])
            if c == 1:
                nc.sync.dma_start(wk_sb[:], wk_v)
        nc.sync.dma_start(bk_sb[:], bk_v)
        nc.sync.dma_start(bq_sb[:], bq_v)
        nc.sync.dma_start(wq_sb[:], wq_v)
        nc.sync.dma_start(xTq_sb[:], xTq_v)
        nc.gpsimd.memset(Q2_sb[:, :, 1, :], 0.0)
        nc.gpsimd.memset(K_sb[:, :, S:], 0.0)
        nc.vector.memset(V_sb[:, :, :, DK:DK + 1], 1.0)

        nc.sync.dma_start(xqb_sb[:], xqb_v)

        # ---------------- constants / biases ----------------
        b1_sb = small.tile([P, NF], F32, tag="b1")
        nc.sync.dma_start(b1_sb[:], b1_v)
        b1x16 = small.tile([P, NF], F32, tag="b1x16")
        nc.vector.tensor_scalar_mul(b1x16[:], b1_sb[:], 16.0)

        # row staging goes through partition 0 of the (not-yet-used) LN
        # squares-dump tile — SBUF is too tight for a dedicated rows pool
        sqd_sb = late.tile([P, D], BF16, tag="sqd")
        nc.sync.dma_start(out1_sb[0:1, 0, :], b2[None, :])
        b2s_bc = small.tile([P, D], BF16, tag="b2s_bc")
        nc.vector.tensor_scalar_mul(b2s_bc[0:1, :], out1_sb[0:1, 0, :], S_FF)
        nc.gpsimd.partition_broadcast(b2s_bc[:], b2s_bc[0:1, :])

        ag_st = late.tile([P, 8], F32, tag="lnstat", bufs=2)
        nc.sync.dma_start(ag_st[0:1, 0:1], alpha[None, :])
        nc.sync.dma_start(ag_st[0:1, 1:2], gamma[None, :])
        ag_bc = small.tile([P, 2], F32, tag="ag_bc")
        nc.gpsimd.partition_broadcast(ag_bc[:], ag_st[0:1, 0:2])
        alpha_bc = ag_bc[:, 0:1]
        gamma_bc = ag_bc[:, 1:2]
        ag256 = small.tile([P, 2], F32, tag="ag256")
        nc.vector.tensor_scalar_mul(ag256[:], ag_bc[:], S_FF)
        alpha256_bc = ag256[:, 0:1]
        gamma256_bc = ag256[:, 1:2]

        eps_bc = small.tile([P, 1], F32, tag="eps_bc")
        nc.vector.memset(eps_bc[:], EPS)
        lnc_bc = small.tile([P, 1], F32, tag="lnc_bc")
        nc.vector.memset(lnc_bc[:], LNC)

        # ================= PSUM pools (8 banks total) =================
        psd_cm = tc.tile_pool(name="psd", bufs=2, space="PSUM")
        psd = psd_cm.__enter__()        # ps4 [P,4,MQ] f32 = 2 banks x2
        psc_cm = tc.tile_pool(name="psc", bufs=2, space="PSUM")
        psc = psc_cm.__enter__()        # c2 [P,512] f32 = 1 bank x2
        pse_cm = tc.tile_pool(name="pse", bufs=2, space="PSUM")
        pse = pse_cm.__enter__()        # pe [P,512] f32 x2 (fillers)

        # ---------------- filler helpers ----------------
        def v_tiles(tt0, n):
            """V projection for token tiles tt0..tt0+n-1 (32*v, no bias)."""
            VN = 512
            for tt in range(tt0, tt0 + n):
                for nv in range(D // VN):
                    ps = pse.tile([P, VN], F32, tag="pe")
                    for kk in range(NJ // 2):
                        nc.tensor.matmul(
                            ps[:], xt_all[:, ds(2 * kk, 2), ts(tt, P)],
                            wv_sb[:, ds(2 * kk, 2), ds(nv * VN, VN)],
                            start=(kk == 0), stop=(kk == NJ // 2 - 1),
                            perf_mode=DR)
                    nc.vector.tensor_copy(
                        V_sb[:, tt, ds(nv * (VN // DK), VN // DK), 0:DK],
                        ps[:].rearrange("p (h d) -> p h d", d=DK))

        def kq_proj(jt):
            for nt in range(NTN):
                ps = pse.tile([P, TN], F32, tag="pe")
                for kk in range(NJ // 2):
                    nc.tensor.matmul(
                        ps[:], wk_sb[:, ds(2 * kk, 2), ts(jt, P)],
                        xt_all[:, ds(2 * kk, 2), ds(nt * TN, TN)],
                        start=(kk == 0), stop=(kk == NJ // 2 - 1),
                        perf_mode=DR)
                nc.vector.tensor_scalar(
                    K_sb[:, jt, ds(nt * TN, TN)], ps[:],
                    1.0 / 16.0, bk_sb[:, jt:jt + 1], ALU.mult, ALU.add)
            ps = pse.tile([P, TQ], F32, tag="pe")
            for kk in range(NJ // 2):
                nc.tensor.matmul(
                    ps[:], wq_sb[:, ds(2 * kk, 2), ts(jt, P)],
                    xTq_sb[:, ds(2 * kk, 2), :],
                    start=(kk == 0), stop=(kk == NJ // 2 - 1),
                    perf_mode=DR)
            nc.vector.tensor_scalar(
                Q2_sb[:, jt, 0, :], ps[:],
                1.0 / 16.0, bq_sb[:, jt:jt + 1], ALU.mult, ALU.add)

        def ln_dve(out_ap, x_ap, sqd_ap, a_bc, g_bc):
            """out = LN(x) via DVE only (x preserved, sqd clobbered)."""
            st = late.tile([P, 8], F32, tag="lnstat", bufs=2)
            nc.vector.reduce_sum(st[:, 0:1], x_ap, axis=AX.X)
            nc.vector.tensor_scalar_mul(st[:, 1:2], st[:, 0:1], 1.0 / D)
            nc.vector.tensor_scalar_mul(st[:, 2:3], st[:, 0:1], -1.0 / D)
            nc.vector.affine_mul_reduce(
                sqd_ap, st[:, 3:4], x_ap, x_ap, 1.0, st[:, 2:3])
            # rstd = 1/sqrt(sum/D + eps)  (tiny Act op, [P,1])
            nc.scalar.activation(st[:, 4:5], st[:, 3:4], ACTF.Sqrt,
                                 scale=1.0 / D, bias=eps_bc)
            nc.vector.reciprocal(st[:, 5:6], st[:, 4:5])
            nc.vector.tensor_tensor(st[:, 6:7], st[:, 5:6], a_bc, ALU.mult)
            # g2 = gamma - mean*k ; out = x*k + g2
            nc.vector.tensor_tensor(st[:, 7:8], st[:, 1:2], st[:, 6:7],
                                    ALU.mult)
            nc.vector.tensor_tensor(st[:, 7:8], g_bc, st[:, 7:8],
                                    ALU.subtract)
            nc.vector.tensor_scalar(out_ap, x_ap, st[:, 6:7], st[:, 7:8],
                                    ALU.mult, ALU.add)

        def e_lns(tts):
            """O-proj + residual + LN1 for query tiles tts (wo streamed)."""
            ON = 512
            reses = {tt: late.tile([P, D], F32, tag="res1", bufs=2,
                                   name=f"res1_{tt}") for tt in tts}
            for no in range(D // ON):
                woc = fstream.tile([P, NJ, ON], FP8, tag="woc", bufs=1,
                                   name=f"wo_{tts[0]}_{no}")
                nc.sync.dma_start(woc[:], wo_v[:, :, ds(no * ON, ON)])
                for tt in tts:
                    pso = pse.tile([P, ON], F32, tag="pe")
                    for kk in range(NJ // 2):
                        nc.tensor.matmul(
                            pso[:], ctx_sb[:, ds(2 * kk, 2), ts(tt, P)],
                            woc[:, ds(2 * kk, 2), :],
                            start=(kk == 0), stop=(kk == NJ // 2 - 1),
                            perf_mode=DR)
                    nc.vector.tensor_tensor(
                        reses[tt][:, ds(no * ON, ON)], pso[:],
                        xqb_sb[:, tt, ds(no * ON, ON)], ALU.add)
            for tt in tts:
                ln_dve(out1_sb[:, tt, :], reses[tt][:], sqd_sb[:],
                       alpha256_bc, gamma256_bc)

        def e_transpose(tt):
            """Transpose out1[tt] into fp8 hi/lo, then pre-add b2 residual."""
            for jt in range(NJ):
                pst = pse.tile([P, 512], F32, tag="pe")
                nc.tensor.transpose(
                    pst[:, 0:P], out1_sb[:, tt, ts(jt, P)], ident[:])
                mid = late.tile([P, P], BF16, tag="tmid", bufs=2)
                nc.vector.tensor_scalar_mul(mid[:], pst[:, 0:P], 1.0 / S_FF)
                nc.gpsimd.tensor_copy(out1T8[:, jt, ts(tt, P)], mid[:])
                nc.gpsimd.tensor_tensor(
                    out1T8l[:, jt, ts(tt, P)], mid[:],
                    out1T8[:, jt, ts(tt, P)], ALU.subtract)
            # out1b = 256*out1 + 256*b2 (FFN2 residual; transposes done)
            nc.gpsimd.tensor_tensor(
                out1_sb[:, tt, :], out1_sb[:, tt, :], b2s_bc[:], ALU.add)

        w1cache = {}

        def f1_chunk(mp, q0, qw):
            """FFN1 for w1 columns [512*mp, 512*mp+512), query cols [q0,q0+qw).
            Streams the w1 hi/lo chunk pair via fstream."""
            whi = fstream.tile([P, NJ, 512], FP8, tag="w1hic",
                               name=f"w1hi_{mp}_{q0}")
            nc.sync.dma_start(whi[:], w1hi_v[:, :, ds(mp * 512, 512)])
            wlo = fstream.tile([P, NJ, 512], FP8, tag="w1loc",
                               name=f"w1lo_{mp}_{q0}")
            nc.sync.dma_start(wlo[:], w1lo_v[:, :, ds(mp * 512, 512)])
            w1cache[mp] = (whi, wlo)
            f1_compute(whi, wlo, mp, q0, qw)

        def f1_compute(whi, wlo, mp, q0, qw):
            for mi in range(4):
                mt = mp * 4 + mi
                ps = pse.tile([P, TN], F32, tag="pe")
                groups = [(whi, out1T8), (wlo, out1T8), (whi, out1T8l)]
                for gi, (wg, xg) in enumerate(groups):
                    for kk in range(NJ // 2):
                        nc.tensor.matmul(
                            ps[:, 0:qw], wg[:, ds(2 * kk, 2), ts(mi, P)],
                            xg[:, ds(2 * kk, 2), ds(q0, qw)],
                            start=(gi == 0 and kk == 0),
                            stop=(gi == 2 and kk == NJ // 2 - 1),
                            perf_mode=DR)
                hmid = fstream.tile([P, TN], BF16, tag="hmid", bufs=2)
                nc.vector.tensor_scalar(
                    hmid[:, 0:qw], ps[:, 0:qw],
                    b1x16[:, mt:mt + 1], 0.0, ALU.add, ALU.max)
                nc.gpsimd.tensor_copy(hid_sb[:, mt, ds(q0, qw)],
                                      hmid[:, 0:qw])
                nc.gpsimd.tensor_tensor(
                    hid_lo[:, mt, ds(q0, qw)], hmid[:, 0:qw],
                    hid_sb[:, mt, ds(q0, qw)], ALU.subtract)

        # ---------------- attention ----------------
        ident = small.tile([P, P], F32, tag="ident")
        from concourse.masks import make_identity
        make_identity(nc, ident)

        fillers = []

        def run_fillers(n):
            for _ in range(n):
                if fillers:
                    fillers.pop(0)()

        def attention_half(half):
            q0 = half * MQ
            for hg in range(H // HG):
                heads = range(hg * HG, (hg + 1) * HG)
                c2s = {h: psc.tile([P, 512], F32, tag="c2",
                       name=f"c2_{half}_{h}") for h in heads}
                exs = {}
                for q4 in range(NQ4 + 1):
                    if q4 < NQ4:
                        for h in heads:
                            hp = (h % HPJ) * DK
                            hj = h // HPJ
                            ps4 = psd.tile([P, 4, MQ], F32, tag="ps4")
                            for i in range(4):
                                mt = q4 * 4 + i
                                nc.tensor.matmul(
                                    ps4[:, i],
                                    K_sb[ds(hp, DK), hj,
                                         ds(mt * P, 2 * P)].rearrange(
                                        "p (u t) -> p u t", u=2),
                                    Q2_sb[ds(hp, DK), hj, :, ds(q0, MQ)],
                                    start=(i % 2 == 0), stop=True,
                                    perf_mode=DR, skip_group_check=True)
                            ex = dpool.tile([P, 4, MQ], FP8, tag="ex",
                                            bufs=4, name=f"ex{half}_{h}_{q4}")
                            nc.scalar.activation(
                                ex[:], ps4[:], ACTF.Exp,
                                scale=1.0 / math.sqrt(DK), bias=lnc_bc[:])
                            exs[(h, q4)] = ex
                    if q4 >= 1:
                        for h in heads:
                            ex = exs.pop((h, q4 - 1))
                            for j in range(2):
                                bp = (q4 - 1) * 2 + j
                                nc.tensor.matmul(
                                    c2s[h][0:DK + 1, 0:MQ],
                                    V_sb[:, ds(2 * bp, 2), h, :],
                                    ex[:, ds(2 * j, 2), :],
                                    start=(bp == 0), stop=(bp == NT // 2 - 1),
                                    perf_mode=DR)
                    run_fillers(2)
                for h in heads:
                    hp = (h % HPJ) * DK
                    hj = h // HPJ
                    recip = dpool.tile([1, MQ], BF16, tag="recip")
                    with nc.allow_low_precision(reason="fp8 ctx"):
                        nc.vector.reciprocal(recip[:],
                                             c2s[h][DK:DK + 1, 0:MQ])
                    recip_bc = dpool.tile([DK, MQ], BF16, tag="recip_bc")
                    nc.gpsimd.partition_broadcast(recip_bc[:], recip[:])
                    # ctx8 = c2/denom = 32*ctx exactly (scales cancel)
                    nc.vector.tensor_tensor(
                        ctx_sb[ds(hp, DK), hj, ds(q0, MQ)],
                        c2s[h][0:DK, 0:MQ], recip_bc[:], ALU.mult)
                run_fillers(2)

        # half A: fillers = V tiles (front-loaded for the attnV lag) + K/Q
        v_tiles(0, 4)      # before kq0: V streams as soon as wv+xt arrive
        kq_proj(0)
        fillers = [lambda tt=tt: v_tiles(2 * tt, 2) for tt in range(2, NT // 2)]
        fillers += [lambda jt=jt: kq_proj(jt) for jt in range(1, NJ)]
        attention_half(0)
        while fillers:
            fillers.pop(0)()

        # release xt + projection weights (dead once K/Q/V are built);
        # the freed space hosts the streamed w2 column-halves
        xtwp_cm.__exit__(None, None, None)
        w2p_cm = tc.tile_pool(name="w2p", bufs=1)
        w2p = w2p_cm.__enter__()
        ON = 512
        w2q = {}

        w2hi_t = {}

        def w2_fetch(no, tag2):
            if no not in w2hi_t:
                whi2 = w2p.tile([P, NF, ON], FP8, tag=f"w2hi{no}")
                nc.sync.dma_start(whi2[:], w2hi_v[:, :, ds(no * ON, ON)])
                w2hi_t[no] = whi2
            wlo2 = w2p.tile([P, NF, ON], FP8, tag="w2lo", name=tag2)
            nc.sync.dma_start(wlo2[:], w2lo_v[:, :, ds(no * ON, ON)])
            w2q[no] = (w2hi_t[no], wlo2)

        def f2_group(tt, no, gsel):
            """FFN2 partial for (tt, no): gsel picks which of the three
            hi/lo groups to emit (PSUM stays open across calls)."""
            whi2, wlo2 = w2q[no]
            groups = [(hid_sb, whi2), (hid_sb, wlo2), (hid_lo, whi2)]
            if gsel == 0:
                f2ps[(tt, no)] = pse.tile([P, ON], F32, tag="pe",
                                          name=f"f2_{tt}_{no}")
            ps = f2ps[(tt, no)]
            lo = 0 if gsel == 0 else gsel + gsel // 2  # 0 -> g0+g1a, 1 -> ...
            for gi, (hg_, wg) in enumerate(groups):
                if gi != gsel:
                    continue
                for kk in range(NF // 2):
                    nc.tensor.matmul(
                        ps[:], hg_[:, ds(2 * kk, 2), ts(tt, P)],
                        wg[:, ds(2 * kk, 2), :],
                        start=(gi == 0 and kk == 0),
                        stop=(gi == 2 and kk == NF // 2 - 1),
                        perf_mode=DR)
            if gsel == 2:
                nc.vector.tensor_tensor(
                    res2_sb[:, tt, ds(no * ON, ON)], ps[:],
                    out1_sb[:, tt, ds(no * ON, ON)], ALU.add)
                del f2ps[(tt, no)]

        f2ps = {}

        def ln2_store(tt):
            o2 = late.tile([P, D], F32, tag="res1", bufs=2,
                           name=f"o2_{tt}")
            ln_dve(o2[:], res2_sb[:, tt, :], sqd_sb[:],
                   alpha_bc, gamma_bc)
            nc.sync.dma_start(out_v[:, tt, :], o2[:])

        # half B: fillers = E(half A) + FFN1(half A); LN1 and its dependent
        # transposes are separate items so PE work never queues behind a
        # DVE chain that hasn't drained yet
        fillers = [lambda: e_lns([0, 1]),
                   lambda: e_transpose(0), lambda: e_transpose(1)]
        fillers += [lambda mp=mp: f1_chunk(mp, 0, MQ)
                    for mp in range(4)]
        fillers += [lambda: w2_fetch(0, "w2lo_a0")]
        fillers += [lambda mp=mp: f1_chunk(mp, 0, MQ)
                    for mp in range(4, DFF // 512)]
        fillers += [lambda tt=tt, g=g: f2_group(tt, 0, g)
                    for tt in (0, 1) for g in (0, 1, 2)]
        fillers += [lambda: w2_fetch(1, "w2lo_a1")]
        fillers += [lambda tt=tt, g=g: f2_group(tt, 1, g)
                    for tt in (0, 1) for g in (0, 1, 2)]
        fillers += [lambda: ln2_store(0), lambda: ln2_store(1)]
        attention_half(1)
        while fillers:
            fillers.pop(0)()

        # ------------- post-B: E(B), FFN1(B), FFN2(B), LN2(B) -------------
        e_lns([2, 3])
        for tt in range(NTQ // 2, NTQ):
            e_transpose(tt)
        # FFN1 for half B: the last two w1 chunk pairs are still resident
        # in fstream's two buffers -> no re-DMA; the rest re-streams
        for mp in (7, 6):
            whi, wlo = w1cache[mp]
            f1_compute(whi, wlo, mp, MQ, MQ)
        for mp in reversed(range(DFF // 512 - 2)):
            f1_chunk(mp, MQ, MQ)

        for tt in range(NTQ // 2, NTQ):
            for g in (0, 1, 2):
                f2_group(tt, 1, g)
        w2_fetch(0, "w2lo_b0")
        for tt in range(NTQ // 2, NTQ):
            for g in (0, 1, 2):
                f2_group(tt, 0, g)
            ln2_store(tt)

        w2p_cm.__exit__(None, None, None)
        dp_cm.__exit__(None, None, None)
        kqv_cm.__exit__(None, None, None)    # release K, Q2, V
        pse_cm.__exit__(None, None, None)
        psc_cm.__exit__(None, None, None)
        psd_cm.__exit__(None, None, None)
        fs_cm.__exit__(None, None, None)
        late_cm.__exit__(None, None, None)
        ctx2_cm.__exit__(None, None, None)

    return nc


_B, _S, _D, _H, _DK, _DFF = 2, 2048, 1024, 16, 64, 4096
_NCORES = 8
_TQ = (_B * _S) // _NCORES    # 512 query tokens per core

_cache = {}


def _get_program():
    if "nc" not in _cache:
        from concourse import bacc
        nc = bacc.Bacc("TRN2", target_bir_lowering=False, debug=False,
                       num_devices=_NCORES)
        build(nc, S=_S, D=_D, H=_H, DK=_DK, DFF=_DFF, TQ=_TQ)
        nc.compile()
        _cache["nc"] = nc
    return _cache["nc"]


def _core_inputs(inp):
    """Host-side prep: per-core input dicts (transposes + fp8 casts only)."""
    import ml_dtypes
    f8 = ml_dtypes.float8_e4m3

    def t8(a, s):
        return np.ascontiguousarray(
            np.asarray(a, np.float32).T * s).astype(f8)

    def hilo(a, s):
        t = np.ascontiguousarray(np.asarray(a, np.float32).T) * s
        hi = t.astype(f8)
        lo = (t - hi.astype(np.float32)).astype(f8)
        return hi, lo

    w1hi, w1lo = hilo(inp["w1"], 16.0)
    w2hi, w2lo = hilo(inp["w2"], 16.0)
    w = {
        "wq8": t8(inp["wq"], 16.0), "wk8": t8(inp["wk"], 16.0),
        "wv8": t8(inp["wv"], 32.0), "wo8": t8(inp["wo"], 32.0),
        "w1hi": w1hi, "w1lo": w1lo, "w2hi": w2hi, "w2lo": w2lo,
        "bq": np.asarray(inp["bq"]), "bk": np.asarray(inp["bk"]),
        "b1": np.asarray(inp["b1"]), "b2": np.asarray(inp["b2"]),
        "alpha": np.asarray(inp["alpha"]), "gamma": np.asarray(inp["gamma"]),
    }
    x = np.asarray(inp["x"], np.float32)
    # bv folded through the O-projection: ctx uses bias-free v, and
    # sum(attn)=1 makes the correction an additive constant bv @ wo^T
    bo = (np.asarray(inp["bo"], np.float32)
          + np.asarray(inp["bv"], np.float32)
          @ np.asarray(inp["wo"], np.float32).T)
    per_batch = _NCORES // _B
    maps = []
    for c in range(_NCORES):
        b, q0 = c // per_batch, (c % per_batch) * _TQ
        xb = x[b]
        m = dict(w)
        m["xT8"] = np.ascontiguousarray(xb.T).astype(f8)
        m["xTq8"] = np.ascontiguousarray(xb[q0:q0 + _TQ].T).astype(f8)
        m["xqb"] = np.ascontiguousarray(
            (xb[q0:q0 + _TQ] + bo) * S_RES).astype(ml_dtypes.bfloat16)
        maps.append(m)
    return maps


def kernel(**inputs) -> np.ndarray:
    from concourse.bass_utils import run_bass_kernel_spmd
    nc = _get_program()
    in_maps = _core_inputs(inputs)
    res = run_bass_kernel_spmd(nc, in_maps, core_ids=list(range(_NCORES)))
    out = np.empty((_B, _S, _D), dtype=np.float32)
    per_batch = _NCORES // _B
    for c, rm in enumerate(res.results):
        b, q0 = c // per_batch, (c % per_batch) * _TQ
        out[b, q0:q0 + _TQ] = rm["out"]
    return out


# revision 87
# speedup vs baseline: 1.1407x; 1.0558x over previous
"""Self-contained Trainium2 Bass kernel for the nn_EnocoderBlock problem.

kernel(**inputs) takes the full (unsharded) inputs of the reference encoder
block (B=2, S=2048, D=1024, H=16, DFF=4096) and returns the full [B, S, D]
fp32 output, running SPMD on 8 NeuronCores.

Sharding: data-parallel over batch x query-token blocks — each of the 8
cores owns one batch element's full K/V context and a 512-token query
slice, so no cross-core collectives are needed.

Precision: all large GEMMs run in fp8e4m3 with DoubleRow perf mode (0.5
PE cycles per output column).  The QK^T scores (64-deep contraction) use
DoubleRow with a zero second slot in the moving operand.  FFN weights
and activations are split into fp8 hi+lo pairs (error compensation), so
the end-to-end error stays ~2e-3.  Scale factors fold into weights / the
exp bias / LayerNorm constants (LayerNorm is scale-invariant).

Schedule: softmax exp on the Activation engine is the critical resource
(~133us).  Attention runs in two query halves; the exp stream starts as
early as possible and everything else (V/K/Q projections in half A; the
O-projection, LayerNorm1, transposes and FFN1 of half A inside half B's
window) is interleaved into the exp-bound windows as PE/DVE/Pool filler.
LayerNorms run DVE-only (affine_mul_reduce) to keep Act pure-exp.
"""

import sys
for _p in ("/opt/trn_rl_repo", "/root/.axon_site/_ro/trn_rl_repo"):
    if _p not in sys.path:
        sys.path.append(_p)

import numpy as np

import math
from contextlib import ExitStack

import concourse.mybir as mybir
import concourse.tile as tile
from concourse.bass import ds, ts

F32 = mybir.dt.float32
BF16 = mybir.dt.bfloat16
FP8 = mybir.dt.float8e4
AX = mybir.AxisListType
ALU = mybir.AluOpType
ACTF = mybir.ActivationFunctionType
DR = mybir.MatmulPerfMode.DoubleRow

P = 128
EPS = 1e-6
LNC = math.log(4.0)       # exp scale constant folded into activation bias
S_RES = 1024.0            # attention residual pre-scale (ctx32 @ wo32)
S_FF = 256.0              # ffn residual pre-scale (hid16 @ w216)


def build(nc, S=2048, D=1024, H=16, DK=64, DFF=4096, TQ=512):
    assert DK == 64 and D % P == 0 and S % P == 0 and DFF % P == 0
    NJ = D // P            # feature tiles of 128 (8)
    NT = S // P            # token tiles of 128 (16)
    NTQ = TQ // P          # query token tiles of 128 (4)
    NF = DFF // P          # dff tiles of 128 (32)
    HPJ = P // DK          # heads per 128-feature tile (2)
    HG = 2                 # attention head-group size
    TN = 512               # moving-dim tile (tokens)
    NTN = S // TN          # 4
    MQ = TQ // 2           # query-half width (256)
    NQ4 = NT // 4          # score quads per head per half (4)

    # ---------------- DRAM I/O ----------------
    def din(name, shape, dt):
        return nc.dram_tensor(name, shape, dt, kind="ExternalInput").ap()

    xT8 = din("xT8", [D, S], FP8)
    xTq8 = din("xTq8", [D, TQ], FP8)
    xqb = din("xqb", [TQ, D], BF16)           # S_RES * (x_q + bo + bv@wo^T)
    wv8, wk8 = din("wv8", [D, D], FP8), din("wk8", [D, D], FP8)
    wq8, wo8 = din("wq8", [D, D], FP8), din("wo8", [D, D], FP8)
    w1hi, w1lo = din("w1hi", [D, DFF], FP8), din("w1lo", [D, DFF], FP8)
    w2hi, w2lo = din("w2hi", [DFF, D], FP8), din("w2lo", [DFF, D], FP8)
    bq, bk = din("bq", [D], F32), din("bk", [D], F32)
    b1, b2 = din("b1", [DFF], F32), din("b2", [D], F32)
    alpha, gamma = din("alpha", [1], F32), din("gamma", [1], F32)
    out = nc.dram_tensor("out", [TQ, D], F32, kind="ExternalOutput").ap()

    xT_v = xT8.rearrange("(o p) t -> p o t", p=P)
    xTq_v = xTq8.rearrange("(o p) t -> p o t", p=P)
    xqb_v = xqb.rearrange("(o p) d -> p o d", p=P)
    out_v = out.rearrange("(o p) d -> p o d", p=P)
    wv_v = wv8.rearrange("(o p) j -> p o j", p=P)
    wk_v = wk8.rearrange("(o p) j -> p o j", p=P)
    wq_v = wq8.rearrange("(o p) j -> p o j", p=P)
    wo_v = wo8.rearrange("(o p) j -> p o j", p=P)
    w1hi_v = w1hi.rearrange("(o p) f -> p o f", p=P)
    w1lo_v = w1lo.rearrange("(o p) f -> p o f", p=P)
    w2hi_v = w2hi.rearrange("(o p) j -> p o j", p=P)
    w2lo_v = w2lo.rearrange("(o p) j -> p o j", p=P)
    bq_v = bq.rearrange("(o p) -> p o", p=P)
    bk_v = bk.rearrange("(o p) -> p o", p=P)
    b1_v = b1.rearrange("(o p) -> p o", p=P)

    with tile.TileContext(nc) as tc, ExitStack() as octx:
        small = octx.enter_context(tc.tile_pool(name="small", bufs=1))

        # ============ pools (LIFO; xtwp dies mid-A, kqv at B-end) ========
        ctx2_cm = tc.tile_pool(name="ctx2", bufs=1)
        ctx2 = ctx2_cm.__enter__()
        ctx_sb = ctx2.tile([P, NJ, TQ], FP8, tag="ctx")
        xqb_sb = ctx2.tile([P, NTQ, D], BF16, tag="xqb")

        late_cm = tc.tile_pool(name="late", bufs=1)
        late = late_cm.__enter__()
        out1_sb = late.tile([P, NTQ, D], F32, tag="out1")    # 256*out1
        out1T8 = late.tile([P, NJ, TQ], FP8, tag="out1T")
        out1T8l = late.tile([P, NJ, TQ], FP8, tag="out1Tl")
        hid_sb = late.tile([P, NF, TQ], FP8, tag="hid")      # 16*relu hi
        hid_lo = late.tile([P, NF, TQ], FP8, tag="hidlo")
        res2_sb = late.tile([P, NTQ, D], BF16, tag="res2")

        fs_cm = tc.tile_pool(name="fstream", bufs=2)
        fstream = fs_cm.__enter__()

        kqv_cm = tc.tile_pool(name="kqv", bufs=1)
        kqv = kqv_cm.__enter__()
        # K has a zeroed 128-token tail: the DR scores lhsT uses 2 token-
        # tile slots and slot 1 (multiplying Q2's zero slot) must be finite
        K_sb = kqv.tile([P, NJ, S + P], FP8, tag="K")
        Q2_sb = kqv.tile([P, NJ, 2, TQ], FP8, tag="Q2")      # [Q; 0] pairs
        V_sb = kqv.tile([P, NT, H, DK + 1], FP8, tag="V")

        dp_cm = tc.tile_pool(name="dpool", bufs=2)
        dpool = dp_cm.__enter__()

        xtwp_cm = tc.tile_pool(name="xtwp", bufs=1)
        xtwp = xtwp_cm.__enter__()
        xt_all = xtwp.tile([P, NJ, S], FP8, tag="xt")
        wv_sb = xtwp.tile([P, NJ, D], FP8, tag="wv")
        wk_sb = xtwp.tile([P, NJ, D], FP8, tag="wk")
        wq_sb = xtwp.tile([P, NJ, D], FP8, tag="wq")
        xTq_sb = xtwp.tile([P, NJ, TQ], FP8, tag="xTq")

        # ---- input DMAs, ordered for streaming ----
        nc.sync.dma_start(wv_sb[:], wv_v)
        bq_sb = small.tile([P, NJ], F32, tag="bq")
        bk_sb = small.tile([P, NJ], F32, tag="bk")
        XC = 512
        for c in range(S // XC):
            nc.sync.dma_start(xt_all[:, :, ds(c * XC, XC)],
                              xT_v[:, :, ds(c * XC, XC)])
            if c == 1:
                nc.sync.dma_start(wk_sb[:], wk_v)
        nc.sync.dma_start(bk_sb[:], bk_v)
        nc.sync.dma_start(bq_sb[:], bq_v)
        nc.sync.dma_start(wq_sb[:], wq_v)
        nc.sync.dma_start(xTq_sb[:], xTq_v)
        nc.gpsimd.memset(Q2_sb[:, :, 1, :], 0.0)
        nc.gpsimd.memset(K_sb[:, :, S:], 0.0)
        nc.vector.memset(V_sb[:, :, :, DK:DK + 1], 1.0)

        nc.sync.dma_start(xqb_sb[:], xqb_v)

        # ---------------- constants / biases ----------------
        b1_sb = small.tile([P, NF], F32, tag="b1")
        nc.sync.dma_start(b1_sb[:], b1_v)
        b1x16 = small.tile([P, NF], F32, tag="b1x16")
        nc.vector.tensor_scalar_mul(b1x16[:], b1_sb[:], 16.0)

        # row staging goes through partition 0 of the (not-yet-used) LN
        # squares-dump tile — SBUF is too tight for a dedicated rows pool
        sqd_sb = late.tile([P, D], BF16, tag="sqd")
        nc.sync.dma_start(out1_sb[0:1, 0, :], b2[None, :])
        b2s_bc = small.tile([P, D], BF16, tag="b2s_bc")
        nc.vector.tensor_scalar_mul(b2s_bc[0:1, :], out1_sb[0:1, 0, :], S_FF)
        nc.gpsimd.partition_broadcast(b2s_bc[:], b2s_bc[0:1, :])

        ag_st = late.tile([P, 8], F32, tag="lnstat", bufs=2)
        nc.sync.dma_start(ag_st[0:1, 0:1], alpha[None, :])
        nc.sync.dma_start(ag_st[0:1, 1:2], gamma[None, :])
        ag_bc = small.tile([P, 2], F32, tag="ag_bc")
        nc.gpsimd.partition_broadcast(ag_bc[:], ag_st[0:1, 0:2])
        alpha_bc = ag_bc[:, 0:1]
        gamma_bc = ag_bc[:, 1:2]
        ag256 = small.tile([P, 2], F32, tag="ag256")
        nc.vector.tensor_scalar_mul(ag256[:], ag_bc[:], S_FF)
        alpha256_bc = ag256[:, 0:1]
        gamma256_bc = ag256[:, 1:2]

        eps_bc = small.tile([P, 1], F32, tag="eps_bc")
        nc.vector.memset(eps_bc[:], EPS)
        lnc_bc = small.tile([P, 1], F32, tag="lnc_bc")
        nc.vector.memset(lnc_bc[:], LNC)

        # ================= PSUM pools (8 banks total) =================
        psd_cm = tc.tile_pool(name="psd", bufs=2, space="PSUM")
        psd = psd_cm.__enter__()        # ps4 [P,4,MQ] f32 = 2 banks x2
        psc_cm = tc.tile_pool(name="psc", bufs=2, space="PSUM")
        psc = psc_cm.__enter__()        # c2 [P,512] f32 = 1 bank x2
        pse_cm = tc.tile_pool(name="pse", bufs=2, space="PSUM")
        pse = pse_cm.__enter__()        # pe [P,512] f32 x2 (fillers)

        # ---------------- filler helpers ----------------
        def v_tiles(tt0, n):
            """V projection for token tiles tt0..tt0+n-1 (32*v, no bias)."""
            VN = 512
            for tt in range(tt0, tt0 + n):
                for nv in range(D // VN):
                    ps = pse.tile([P, VN], F32, tag="pe")
                    for kk in range(NJ // 2):
                        nc.tensor.matmul(
                            ps[:], xt_all[:, ds(2 * kk, 2), ts(tt, P)],
                            wv_sb[:, ds(2 * kk, 2), ds(nv * VN, VN)],
                            start=(kk == 0), stop=(kk == NJ // 2 - 1),
                            perf_mode=DR)
                    nc.vector.tensor_copy(
                        V_sb[:, tt, ds(nv * (VN // DK), VN // DK), 0:DK],
                        ps[:].rearrange("p (h d) -> p h d", d=DK))

        def kq_k_half(jt, h2):
            for nt in range(2 * h2, 2 * h2 + 2):
                ps = pse.tile([P, TN], F32, tag="pe")
                for kk in range(NJ // 2):
                    nc.tensor.matmul(
                        ps[:], wk_sb[:, ds(2 * kk, 2), ts(jt, P)],
                        xt_all[:, ds(2 * kk, 2), ds(nt * TN, TN)],
                        start=(kk == 0), stop=(kk == NJ // 2 - 1),
                        perf_mode=DR)
                nc.vector.tensor_scalar(
                    K_sb[:, jt, ds(nt * TN, TN)], ps[:],
                    1.0 / 16.0, bk_sb[:, jt:jt + 1], ALU.mult, ALU.add)

        def kq_q(jt):
            ps = pse.tile([P, TQ], F32, tag="pe")
            for kk in range(NJ // 2):
                nc.tensor.matmul(
                    ps[:], wq_sb[:, ds(2 * kk, 2), ts(jt, P)],
                    xTq_sb[:, ds(2 * kk, 2), :],
                    start=(kk == 0), stop=(kk == NJ // 2 - 1),
                    perf_mode=DR)
            nc.vector.tensor_scalar(
                Q2_sb[:, jt, 0, :], ps[:],
                1.0 / 16.0, bq_sb[:, jt:jt + 1], ALU.mult, ALU.add)

        def kq_proj(jt):
            kq_k_half(jt, 0)
            kq_k_half(jt, 1)
            kq_q(jt)

        def ln_dve(out_ap, x_ap, sqd_ap, a_bc, g_bc):
            """out = LN(x) via DVE only (x preserved, sqd clobbered)."""
            st = late.tile([P, 8], F32, tag="lnstat", bufs=2)
            nc.vector.reduce_sum(st[:, 0:1], x_ap, axis=AX.X)
            nc.vector.tensor_scalar_mul(st[:, 1:2], st[:, 0:1], 1.0 / D)
            nc.vector.tensor_scalar_mul(st[:, 2:3], st[:, 0:1], -1.0 / D)
            nc.vector.affine_mul_reduce(
                sqd_ap, st[:, 3:4], x_ap, x_ap, 1.0, st[:, 2:3])
            # rstd = 1/sqrt(sum/D + eps)  (tiny Act op, [P,1])
            nc.scalar.activation(st[:, 4:5], st[:, 3:4], ACTF.Sqrt,
                                 scale=1.0 / D, bias=eps_bc)
            nc.vector.reciprocal(st[:, 5:6], st[:, 4:5])
            nc.vector.tensor_tensor(st[:, 6:7], st[:, 5:6], a_bc, ALU.mult)
            # g2 = gamma - mean*k ; out = x*k + g2
            nc.vector.tensor_tensor(st[:, 7:8], st[:, 1:2], st[:, 6:7],
                                    ALU.mult)
            nc.vector.tensor_tensor(st[:, 7:8], g_bc, st[:, 7:8],
                                    ALU.subtract)
            nc.vector.tensor_scalar(out_ap, x_ap, st[:, 6:7], st[:, 7:8],
                                    ALU.mult, ALU.add)

        def e_lns(tts):
            """O-proj + residual + LN1 for query tiles tts (wo streamed)."""
            ON = 512
            reses = {tt: late.tile([P, D], F32, tag="res1", bufs=2,
                                   name=f"res1_{tt}") for tt in tts}
            for no in range(D // ON):
                woc = fstream.tile([P, NJ, ON], FP8, tag="woc", bufs=1,
                                   name=f"wo_{tts[0]}_{no}")
                nc.sync.dma_start(woc[:], wo_v[:, :, ds(no * ON, ON)])
                for tt in tts:
                    pso = pse.tile([P, ON], F32, tag="pe")
                    for kk in range(NJ // 2):
                        nc.tensor.matmul(
                            pso[:], ctx_sb[:, ds(2 * kk, 2), ts(tt, P)],
                            woc[:, ds(2 * kk, 2), :],
                            start=(kk == 0), stop=(kk == NJ // 2 - 1),
                            perf_mode=DR)
                    nc.vector.tensor_tensor(
                        reses[tt][:, ds(no * ON, ON)], pso[:],
                        xqb_sb[:, tt, ds(no * ON, ON)], ALU.add)
            for tt in tts:
                ln_dve(out1_sb[:, tt, :], reses[tt][:], sqd_sb[:],
                       alpha256_bc, gamma256_bc)

        def e_transpose(tt):
            """Transpose out1[tt] into fp8 hi/lo, then pre-add b2 residual."""
            for jt in range(NJ):
                pst = pse.tile([P, 512], F32, tag="pe")
                nc.tensor.transpose(
                    pst[:, 0:P], out1_sb[:, tt, ts(jt, P)], ident[:])
                mid = late.tile([P, P], BF16, tag="tmid", bufs=2)
                nc.vector.tensor_scalar_mul(mid[:], pst[:, 0:P], 1.0 / S_FF)
                nc.gpsimd.tensor_copy(out1T8[:, jt, ts(tt, P)], mid[:])
                nc.gpsimd.tensor_tensor(
                    out1T8l[:, jt, ts(tt, P)], mid[:],
                    out1T8[:, jt, ts(tt, P)], ALU.subtract)
            # out1b = 256*out1 + 256*b2 (FFN2 residual; transposes done)
            nc.gpsimd.tensor_tensor(
                out1_sb[:, tt, :], out1_sb[:, tt, :], b2s_bc[:], ALU.add)

        w1cache = {}

        def f1_fetch(mp, q0):
            whi = fstream.tile([P, NJ, 512], FP8, tag="w1hic",
                               name=f"w1hi_{mp}_{q0}")
            nc.sync.dma_start(whi[:], w1hi_v[:, :, ds(mp * 512, 512)])
            wlo = fstream.tile([P, NJ, 512], FP8, tag="w1loc",
                               name=f"w1lo_{mp}_{q0}")
            nc.sync.dma_start(wlo[:], w1lo_v[:, :, ds(mp * 512, 512)])
            w1cache[mp] = (whi, wlo)
            return whi, wlo

        def f1_pair(mp, mi2, q0, qw):
            """FFN1 for two of the four mt tiles of w1 chunk mp."""
            if mi2 == 0:
                whi, wlo = f1_fetch(mp, q0)
            else:
                whi, wlo = w1cache[mp]
            f1_compute(whi, wlo, mp, q0, qw, mi0=2 * mi2, mi1=2 * mi2 + 2)

        def f1_chunk(mp, q0, qw):
            whi, wlo = f1_fetch(mp, q0)
            f1_compute(whi, wlo, mp, q0, qw)

        def f1_compute(whi, wlo, mp, q0, qw, mi0=0, mi1=4):
            for mi in range(mi0, mi1):
                mt = mp * 4 + mi
                ps = pse.tile([P, TN], F32, tag="pe")
                groups = [(whi, out1T8), (wlo, out1T8), (whi, out1T8l)]
                for gi, (wg, xg) in enumerate(groups):
                    for kk in range(NJ // 2):
                        nc.tensor.matmul(
                            ps[:, 0:qw], wg[:, ds(2 * kk, 2), ts(mi, P)],
                            xg[:, ds(2 * kk, 2), ds(q0, qw)],
                            start=(gi == 0 and kk == 0),
                            stop=(gi == 2 and kk == NJ // 2 - 1),
                            perf_mode=DR)
                hmid = fstream.tile([P, TN], BF16, tag="hmid", bufs=2)
                nc.vector.tensor_scalar(
                    hmid[:, 0:qw], ps[:, 0:qw],
                    b1x16[:, mt:mt + 1], 0.0, ALU.add, ALU.max)
                nc.gpsimd.tensor_copy(hid_sb[:, mt, ds(q0, qw)],
                                      hmid[:, 0:qw])
                nc.gpsimd.tensor_tensor(
                    hid_lo[:, mt, ds(q0, qw)], hmid[:, 0:qw],
                    hid_sb[:, mt, ds(q0, qw)], ALU.subtract)

        # ---------------- attention ----------------
        ident = small.tile([P, P], F32, tag="ident")
        from concourse.masks import make_identity
        make_identity(nc, ident)

        fillers = []

        def run_fillers(n):
            for _ in range(n):
                if fillers:
                    f = fillers.pop(0)
                    if f is not None:
                        f()

        def attention_half(half):
            q0 = half * MQ
            for hg in range(H // HG):
                heads = range(hg * HG, (hg + 1) * HG)
                c2s = {h: psc.tile([P, 512], F32, tag="c2",
                       name=f"c2_{half}_{h}") for h in heads}
                exs = {}
                for q4 in range(NQ4 + 1):
                    if q4 < NQ4:
                        for h in heads:
                            hp = (h % HPJ) * DK
                            hj = h // HPJ
                            ps4 = psd.tile([P, 4, MQ], F32, tag="ps4")
                            for i in range(4):
                                mt = q4 * 4 + i
                                nc.tensor.matmul(
                                    ps4[:, i],
                                    K_sb[ds(hp, DK), hj,
                                         ds(mt * P, 2 * P)].rearrange(
                                        "p (u t) -> p u t", u=2),
                                    Q2_sb[ds(hp, DK), hj, :, ds(q0, MQ)],
                                    start=(i % 2 == 0), stop=True,
                                    perf_mode=DR, skip_group_check=True)
                            ex = dpool.tile([P, 4, MQ], FP8, tag="ex",
                                            bufs=4, name=f"ex{half}_{h}_{q4}")
                            nc.scalar.activation(
                                ex[:], ps4[:], ACTF.Exp,
                                scale=1.0 / math.sqrt(DK), bias=lnc_bc[:])
                            exs[(h, q4)] = ex
                    if q4 >= 1:
                        for h in heads:
                            ex = exs.pop((h, q4 - 1))
                            for j in range(2):
                                bp = (q4 - 1) * 2 + j
                                nc.tensor.matmul(
                                    c2s[h][0:DK + 1, 0:MQ],
                                    V_sb[:, ds(2 * bp, 2), h, :],
                                    ex[:, ds(2 * j, 2), :],
                                    start=(bp == 0), stop=(bp == NT // 2 - 1),
                                    perf_mode=DR)
                    run_fillers(1)
                for h in heads:
                    hp = (h % HPJ) * DK
                    hj = h // HPJ
                    recip = dpool.tile([1, MQ], BF16, tag="recip")
                    with nc.allow_low_precision(reason="fp8 ctx"):
                        nc.vector.reciprocal(recip[:],
                                             c2s[h][DK:DK + 1, 0:MQ])
                    recip_bc = dpool.tile([DK, MQ], BF16, tag="recip_bc")
                    nc.gpsimd.partition_broadcast(recip_bc[:], recip[:])
                    # ctx8 = c2/denom = 32*ctx exactly (scales cancel)
                    nc.vector.tensor_tensor(
                        ctx_sb[ds(hp, DK), hj, ds(q0, MQ)],
                        c2s[h][0:DK, 0:MQ], recip_bc[:], ALU.mult)
                run_fillers(2)

        # half A: fillers = V tiles (front-loaded for the attnV lag) + K/Q
        v_tiles(0, 8)      # in the startup DMA shadow (needs only wv+xt c0-1)
        kq_proj(0)
        fillers = [lambda tt=tt: v_tiles(2 * tt, 2) for tt in range(4, 8)]
        for jt in range(1, NJ):
            fillers += [lambda jt=jt: kq_k_half(jt, 0),
                        lambda jt=jt: kq_k_half(jt, 1),
                        lambda jt=jt: kq_q(jt)]
        attention_half(0)
        while fillers:
            f = fillers.pop(0)
            if f is not None:
                f()

        # release xt + projection weights (dead once K/Q/V are built);
        # the freed space hosts the streamed w2 column-halves
        xtwp_cm.__exit__(None, None, None)
        w2p_cm = tc.tile_pool(name="w2p", bufs=1)
        w2p = w2p_cm.__enter__()
        ON = 512
        w2q = {}

        w2hi_t = {}

        def w2_fetch(no, tag2):
            if no not in w2hi_t:
                whi2 = w2p.tile([P, NF, ON], FP8, tag=f"w2hi{no}")
                nc.sync.dma_start(whi2[:], w2hi_v[:, :, ds(no * ON, ON)])
                w2hi_t[no] = whi2
            wlo2 = w2p.tile([P, NF, ON], FP8, tag="w2lo", name=tag2)
            nc.sync.dma_start(wlo2[:], w2lo_v[:, :, ds(no * ON, ON)])
            w2q[no] = (w2hi_t[no], wlo2)

        def f2_group(tt, no, gsel):
            """FFN2 partial for (tt, no): gsel picks which of the three
            hi/lo groups to emit (PSUM stays open across calls)."""
            whi2, wlo2 = w2q[no]
            groups = [(hid_sb, whi2), (hid_sb, wlo2), (hid_lo, whi2)]
            if gsel == 0:
                f2ps[(tt, no)] = pse.tile([P, ON], F32, tag="pe",
                                          name=f"f2_{tt}_{no}")
            ps = f2ps[(tt, no)]
            lo = 0 if gsel == 0 else gsel + gsel // 2  # 0 -> g0+g1a, 1 -> ...
            for gi, (hg_, wg) in enumerate(groups):
                if gi != gsel:
                    continue
                for kk in range(NF // 2):
                    nc.tensor.matmul(
                        ps[:], hg_[:, ds(2 * kk, 2), ts(tt, P)],
                        wg[:, ds(2 * kk, 2), :],
                        start=(gi == 0 and kk == 0),
                        stop=(gi == 2 and kk == NF // 2 - 1),
                        perf_mode=DR)
            if gsel == 2:
                nc.vector.tensor_tensor(
                    res2_sb[:, tt, ds(no * ON, ON)], ps[:],
                    out1_sb[:, tt, ds(no * ON, ON)], ALU.add)
                del f2ps[(tt, no)]

        f2ps = {}

        def ln2_store(tt):
            o2 = late.tile([P, D], F32, tag="res1", bufs=2,
                           name=f"o2_{tt}")
            ln_dve(o2[:], res2_sb[:, tt, :], sqd_sb[:],
                   alpha_bc, gamma_bc)
            nc.sync.dma_start(out_v[:, tt, :], o2[:])

        # half B: fillers = E(half A) + FFN1(half A); LN1 and its dependent
        # transposes are separate items so PE work never queues behind a
        # DVE chain that hasn't drained yet
        fillers = [lambda: e_lns([0, 1]), lambda: w2_fetch(0, "w2lo_a0"),
                   None, None,
                   lambda: e_transpose(0), lambda: e_transpose(1)]
        fillers += [lambda mp=mp, mi=mi: f1_pair(mp, mi, 0, MQ)
                    for mp in range(DFF // 512) for mi in (0, 1)]
        fillers += [lambda tt=tt, g=g: f2_group(tt, 0, g)
                    for tt in (0, 1) for g in (0, 1, 2)]
        fillers += [lambda: w2_fetch(1, "w2lo_a1")]
        fillers += [lambda tt=tt, g=g: f2_group(tt, 1, g)
                    for tt in (0, 1) for g in (0, 1, 2)]
        fillers += [lambda: ln2_store(0), lambda: ln2_store(1)]
        attention_half(1)
        while fillers:
            fillers.pop(0)()

        # ------------- post-B: E(B), FFN1(B), FFN2(B), LN2(B) -------------
        e_lns([2, 3])
        for tt in range(NTQ // 2, NTQ):
            e_transpose(tt)
        # FFN1 for half B: the last two w1 chunk pairs are still resident
        # in fstream's two buffers -> no re-DMA; the rest re-streams
        for mp in (7, 6):
            whi, wlo = w1cache[mp]
            f1_compute(whi, wlo, mp, MQ, MQ)
        for mp in reversed(range(DFF // 512 - 2)):
            f1_chunk(mp, MQ, MQ)

        for tt in range(NTQ // 2, NTQ):
            for g in (0, 1, 2):
                f2_group(tt, 1, g)
        w2_fetch(0, "w2lo_b0")
        for tt in range(NTQ // 2, NTQ):
            for g in (0, 1, 2):
                f2_group(tt, 0, g)
            ln2_store(tt)

        w2p_cm.__exit__(None, None, None)
        dp_cm.__exit__(None, None, None)
        kqv_cm.__exit__(None, None, None)    # release K, Q2, V
        pse_cm.__exit__(None, None, None)
        psc_cm.__exit__(None, None, None)
        psd_cm.__exit__(None, None, None)
        fs_cm.__exit__(None, None, None)
        late_cm.__exit__(None, None, None)
        ctx2_cm.__exit__(None, None, None)

    return nc


_B, _S, _D, _H, _DK, _DFF = 2, 2048, 1024, 16, 64, 4096
_NCORES = 8
_TQ = (_B * _S) // _NCORES    # 512 query tokens per core

_cache = {}


def _get_program():
    if "nc" not in _cache:
        from concourse import bacc
        nc = bacc.Bacc("TRN2", target_bir_lowering=False, debug=False,
                       num_devices=_NCORES)
        build(nc, S=_S, D=_D, H=_H, DK=_DK, DFF=_DFF, TQ=_TQ)
        nc.compile()
        _cache["nc"] = nc
    return _cache["nc"]


def _core_inputs(inp):
    """Host-side prep: per-core input dicts (transposes + fp8 casts only)."""
    import ml_dtypes
    f8 = ml_dtypes.float8_e4m3

    def t8(a, s):
        return np.ascontiguousarray(
            np.asarray(a, np.float32).T * s).astype(f8)

    def hilo(a, s):
        t = np.ascontiguousarray(np.asarray(a, np.float32).T) * s
        hi = t.astype(f8)
        lo = (t - hi.astype(np.float32)).astype(f8)
        return hi, lo

    w1hi, w1lo = hilo(inp["w1"], 16.0)
    w2hi, w2lo = hilo(inp["w2"], 16.0)
    w = {
        "wq8": t8(inp["wq"], 16.0), "wk8": t8(inp["wk"], 16.0),
        "wv8": t8(inp["wv"], 32.0), "wo8": t8(inp["wo"], 32.0),
        "w1hi": w1hi, "w1lo": w1lo, "w2hi": w2hi, "w2lo": w2lo,
        "bq": np.asarray(inp["bq"]), "bk": np.asarray(inp["bk"]),
        "b1": np.asarray(inp["b1"]), "b2": np.asarray(inp["b2"]),
        "alpha": np.asarray(inp["alpha"]), "gamma": np.asarray(inp["gamma"]),
    }
    x = np.asarray(inp["x"], np.float32)
    # bv folded through the O-projection: ctx uses bias-free v, and
    # sum(attn)=1 makes the correction an additive constant bv @ wo^T
    bo = (np.asarray(inp["bo"], np.float32)
          + np.asarray(inp["bv"], np.float32)
          @ np.asarray(inp["wo"], np.float32).T)
    per_batch = _NCORES // _B
    maps = []
    for c in range(_NCORES):
        b, q0 = c // per_batch, (c % per_batch) * _TQ
        xb = x[b]
        m = dict(w)
        m["xT8"] = np.ascontiguousarray(xb.T).astype(f8)
        m["xTq8"] = np.ascontiguousarray(xb[q0:q0 + _TQ].T).astype(f8)
        m["xqb"] = np.ascontiguousarray(
            (xb[q0:q0 + _TQ] + bo) * S_RES).astype(ml_dtypes.bfloat16)
        maps.append(m)
    return maps


def kernel(**inputs) -> np.ndarray:
    from concourse.bass_utils import run_bass_kernel_spmd
    nc = _get_program()
    in_maps = _core_inputs(inputs)
    res = run_bass_kernel_spmd(nc, in_maps, core_ids=list(range(_NCORES)))
    out = np.empty((_B, _S, _D), dtype=np.float32)
    per_batch = _NCORES // _B
    for c, rm in enumerate(res.results):
        b, q0 = c // per_batch, (c % per_batch) * _TQ
        out[b, q0:q0 + _TQ] = rm["out"]
    return out
